# revision 61
# baseline (speedup 1.0000x reference)
"""GCN encoder (dense+relu -> GCNConv -> {mu, logstd} GCNConv) on 8 Trainium2
NeuronCores.

Strategy (v2):
  - Nodes sharded across 8 cores (12500 rows each, padded to 12544 = 98*128).
  - conv1's message traffic in fp8e4 (gathered rows 256B/edge, scatter
    matrices S, AllGathered table); conv2's messages in f16 with S still fp8
    in DRAM (DVE-converted on load). Error budget: conv1's quantization noise
    is attenuated by conv2's neighborhood averaging; conv2's would hit the
    output directly. PSUM accumulation is f32; dense transforms f16.
  - Edges partitioned by (dest group of GW windows, source chunk); slots
    padded to 128 per (group, chunk) cell only. A tile whose 128 edges span
    several dest windows gets one S block per window it touches (union span
    across cores so the schedule is core-independent). conv1 GW=7, conv2
    GW=2 (f16 tiles are 2x bigger in SBUF).
  - Pad slots carry idx -1 (trailing within each gather call) and the true
    per-core valid count is fed via a runtime register, so the Q7 gather
    ucode skips them entirely (its time is the kernel's hard floor at
    ~2.9ns per real edge, ~1.23ms total).
  - u tables AllGathered in 4 window-aligned pieces (25/25/24/24 row tiles)
    so the next conv's gathers start as soon as their chunk's piece lands.
    AG triggers are deferred one group so their waits never stall gathers
    queued behind them on the gpsimd queue.
  - Degree vectors (dinv, dinv^2, sqrt(deg)) are host-computed inputs; bias
    adds use a rank-1 matmul with the sqrt(deg) row so the per-partition
    output scale folds to the right per-term factors.
  - Dense layer uses host-pretransposed f16 lhsT tiles (no PE transposes),
    4-tile batched loads/stores.
  - Window loop software-pipelined (scatter of w+1 issued before transform
    of w) so PE doesn't stall on the PSUM->SBUF copy round trip; self-loop
    matmul goes last so its DRAM load hides behind the scatter matmuls.
"""
import sys

sys.path.insert(0, "/opt/trn_rl_repo")

import numpy as np
import ml_dtypes

import concourse.bacc as bacc
import concourse.bass as bass
import concourse.mybir as mybir
from concourse import tile
from concourse.bass_utils import run_bass_kernel_spmd
from concourse.masks import make_identity

F32 = mybir.dt.float32
F16 = mybir.dt.float16
F8 = mybir.dt.float8e4
I16 = mybir.dt.int16
NP_F8 = ml_dtypes.float8_e4m3
NP_F16 = np.float16
PAD_TRIM = True
I32 = mybir.dt.int32


class Cfg:
    def __init__(self, N=100000, NC=8, DIN=256, DMID=256, DOUT=256,
                 GW1=6, GW2=2):
        assert N % NC == 0
        self.N, self.NC = N, NC
        self.DIN, self.DMID, self.DOUT = DIN, DMID, DOUT
        self.SH = N // NC                       # real rows per shard
        self.SHP = -(-self.SH // 128) * 128     # padded rows per shard
        self.W = 128                            # dest window size
        self.NW = self.SHP // self.W            # windows (= row tiles)
        self.GW1, self.GW2 = GW1, GW2
        # window-aligned AllGather pieces; piece == gather chunk. 8 small
        # pieces shrink the phase-boundary stalls (the next conv's chunk-c
        # gathers start as soon as AG piece c lands, and the last AG tail
        # halves).
        self.NPIECE = 8
        q, r = divmod(self.NW, self.NPIECE)
        self.piece_tiles = [q + (1 if i < r else 0)
                            for i in range(self.NPIECE)]
        self.piece_rows = [t * 128 for t in self.piece_tiles]
        self.piece_starts = np.concatenate(
            [[0], np.cumsum(self.piece_rows)]).astype(np.int64)
        self.NCHUNK = self.NPIECE
        self.CH = [NC * r for r in self.piece_rows]   # rows per gather chunk
        assert max(self.CH) <= 32767


def _edge_schedule(core, ldst, cidx, chunk, ew, cfg: Cfg, GW, npdt):
    """Core-independent tile/block schedule + per-core S and idx tables."""
    NC, W, NW, NCHUNK = cfg.NC, cfg.W, cfg.NW, cfg.NCHUNK
    NG = -(-NW // GW)
    win = ldst // W
    grp = win // GW
    wig = win - grp * GW
    dstoff = ldst - win * W

    cell = (core * NG + grp) * NCHUNK + chunk
    order = np.lexsort((cidx, wig, cell))
    cell_s = cell[order]
    n_cells = NC * NG * NCHUNK
    counts = np.bincount(cell, minlength=n_cells).reshape(NC, NG, NCHUNK)

    T_gc = -(-counts.max(axis=0) // 128)            # [NG, NCHUNK]
    c_off = np.zeros((NG, NCHUNK), np.int64)
    c_off[:, 1:] = np.cumsum(T_gc, axis=1)[:, :-1]
    tg = T_gc.sum(axis=1)
    base_g = np.zeros(NG, np.int64)
    base_g[1:] = np.cumsum(tg)[:-1]
    TOT = int(tg.sum())
    tile_base = base_g[:, None] + c_off

    starts = np.zeros(n_cells + 1, np.int64)
    starts[1:] = np.cumsum(counts.reshape(-1))
    rank_s = np.arange(len(cell_s), dtype=np.int64) - starts[cell_s]
    g_s = (cell_s // NCHUNK) % NG
    c_s = cell_s % NCHUNK
    core_s = cell_s // (NG * NCHUNK)
    erow_s = (tile_base[g_s, c_s] + rank_s // 128) * 128 + rank_s % 128

    # pad slots get idx -1: they are trailing within each (g,c) gather call,
    # and the Q7 gather kernel trims trailing negative indices (no descriptor
    # work, no DMA bytes). Their S entries are 0 so stale msg data is killed.
    PAD_IDX = -1 if PAD_TRIM else 0
    IDXRAW = np.full((NC, TOT * 128), PAD_IDX, np.int16)
    IDXRAW[core_s, erow_s] = cidx[order].astype(np.int16)
    idxg = IDXRAW.reshape(NC, TOT * 8, 16).transpose(0, 2, 1)
    IDXG = np.tile(idxg, (1, 8, 1))                 # [NC, 128, TOT*8]

    key4 = cell * GW + wig
    counts4 = np.bincount(key4, minlength=n_cells * GW) \
        .reshape(NC, NG, NCHUNK, GW)
    ends4 = np.cumsum(counts4, axis=3)
    starts4 = ends4 - counts4
    has = counts4 > 0
    T_LO = np.where(has, starts4 // 128, 1 << 30).min(axis=0)
    T_HI = np.where(has, (ends4 - 1) // 128, -1).max(axis=0)
    present = has.any(axis=0)

    blk_start = np.full((NG, NCHUNK, GW), -1, np.int64)
    base_blk = np.zeros(NG, np.int64)
    sched = []
    nblk_total = 0
    nidx_cols = []                     # per gather call: per-core valid count
    for g in range(NG):
        ws = list(range(g * GW, min((g + 1) * GW, NW)))
        base_blk[g] = nblk_total
        call_base = len(nidx_cols)
        for c in range(NCHUNK):
            if T_gc[g, c] > 0:
                nidx_cols.append(counts[:, g, c])
        win_blocks = []
        bi = 0
        for wi in range(len(ws)):
            blocks = []
            for c in range(NCHUNK):
                if not present[g, c, wi]:
                    continue
                blk_start[g, c, wi] = bi
                for t in range(int(T_LO[g, c, wi]), int(T_HI[g, c, wi]) + 1):
                    blocks.append((int(c_off[g, c] + t), bi))
                    bi += 1
            win_blocks.append(blocks)
        nblk_total += bi
        sched.append(dict(ws=ws, base_msg=int(base_g[g]), tg=int(tg[g]),
                          c_off=[int(v) for v in c_off[g]],
                          base_blk=int(base_blk[g]), nblk=bi,
                          call_base=call_base,
                          win_blocks=win_blocks))
    B_TOT = nblk_total
    NIDX = np.stack(nidx_cols, axis=1).astype(np.int32)   # [NC, ncalls]

    t_in_cell = rank_s // 128
    wig_s = wig[order]
    blk_s = (base_blk[g_s] + blk_start[g_s, c_s, wig_s]
             + (t_in_cell - T_LO[g_s, c_s, wig_s]))
    Sv = np.zeros((NC, 128, B_TOT * 128), npdt)
    Sv[core_s, rank_s % 128, blk_s * 128 + dstoff[order]] = \
        ew[order].astype(npdt)

    return dict(TOT=TOT, B_TOT=B_TOT, sched=sched, sv=Sv, gidx=IDXG,
                nidx=NIDX)


def _preprocess(x, edge_index, edge_attr, cfg: Cfg):
    NC, SH, SHP = cfg.NC, cfg.SH, cfg.SHP

    src = np.asarray(edge_index[0], dtype=np.int64)
    dst = np.asarray(edge_index[1], dtype=np.int64)
    ew = np.asarray(edge_attr, dtype=np.float32)
    core = dst // SH
    ldst = dst - core * SH
    s_shard = src // SH
    s_loc = src - s_shard * SH
    s_piece = np.searchsorted(cfg.piece_starts, s_loc, side="right") - 1
    prow = np.asarray(cfg.piece_rows, np.int64)
    cidx = s_shard * prow[s_piece] + (s_loc - cfg.piece_starts[s_piece])

    m1 = _edge_schedule(core, ldst, cidx, s_piece, ew, cfg, cfg.GW1, NP_F8)
    m2 = _edge_schedule(core, ldst, cidx, s_piece, ew, cfg, cfg.GW2, NP_F8)

    # degree (with self-loop weight 1) computed on host
    deg = np.bincount(core * SHP + ldst, weights=ew.astype(np.float64),
                      minlength=NC * SHP).reshape(NC, SHP) \
        .astype(np.float32) + 1.0
    dinv2 = 1.0 / deg
    dinv = np.sqrt(dinv2)
    sqd = np.sqrt(deg)
    NW = cfg.NW
    # [NC, 128, NW] layout: [c, p, rt] = value at row rt*128+p
    dinv_t = dinv.reshape(NC, NW, 128).transpose(0, 2, 1).copy()
    dinv2_t = dinv2.reshape(NC, NW, 128).transpose(0, 2, 1).copy()
    sqdrow = sqd.reshape(NC, 1, SHP).astype(NP_F8)

    # dense lhsT tiles, host-transposed, f16:
    # xtb[c, rt*128+p, h*128+j] = x[c*SH + rt*128 + j, h*128 + p]
    xs = np.zeros((NC, SHP, cfg.DIN), np.float32)
    xs[:, :SH, :] = np.asarray(x, np.float32).reshape(NC, SH, cfg.DIN)
    xtb = xs.reshape(NC, NW, 128, cfg.DIN).transpose(0, 1, 3, 2) \
        .reshape(NC, NW, 2, 128, 128).transpose(0, 1, 3, 2, 4) \
        .reshape(NC, NW * 128, cfg.DIN).astype(NP_F16)

    meta = dict(m1=m1, m2=m2)
    data = dict(xtb=xtb, dinv=dinv_t, dinv2=dinv2_t, sqdrow=sqdrow)
    return meta, data


def _build_program(cfg: Cfg, meta):
    NC, SHP, W, NW = cfg.NC, cfg.SHP, cfg.W, cfg.NW
    DIN, DMID, DOUT = cfg.DIN, cfg.DMID, cfg.DOUT
    NCHUNK = cfg.NCHUNK
    m1, m2 = meta["m1"], meta["m2"]
    NRT = NW
    ptiles = cfg.piece_tiles
    pstart_t = np.concatenate([[0], np.cumsum(ptiles)])

    nc = bacc.Bacc("TRN2", target_bir_lowering=False, debug=False,
                   num_devices=NC, num_swdge_queues=4)

    xtb = nc.dram_tensor("xtb", [SHP, DIN], F16, kind="ExternalInput")
    dinv_d = nc.dram_tensor("dinv_d", [128, NRT], F32, kind="ExternalInput")
    dinv2_d = nc.dram_tensor("dinv2_d", [128, NRT], F32, kind="ExternalInput")
    sqdrow_d = nc.dram_tensor("sqdrow_d", [1, SHP], F8, kind="ExternalInput")
    sv1 = nc.dram_tensor("sv1", [128, m1["B_TOT"] * 128], F8,
                         kind="ExternalInput")
    gidx1 = nc.dram_tensor("gidx1", [128, m1["TOT"] * 8], I16,
                           kind="ExternalInput")
    sv2 = nc.dram_tensor("sv2", [128, m2["B_TOT"] * 128], F8,
                         kind="ExternalInput")
    gidx2 = nc.dram_tensor("gidx2", [128, m2["TOT"] * 8], I16,
                           kind="ExternalInput")
    nidx1 = nc.dram_tensor("nidx1", [1, m1["nidx"].shape[1]], I32,
                           kind="ExternalInput")
    nidx2 = nc.dram_tensor("nidx2", [1, m2["nidx"].shape[1]], I32,
                           kind="ExternalInput")
    wd = nc.dram_tensor("wd", [DIN, DMID], F16, kind="ExternalInput")
    bd = nc.dram_tensor("bd", [1, DMID], F32, kind="ExternalInput")
    we = nc.dram_tensor("we", [DMID, DMID], F16, kind="ExternalInput")
    be = nc.dram_tensor("be", [1, DMID], F8, kind="ExternalInput")
    wc = nc.dram_tensor("wc", [DMID, DOUT], F16, kind="ExternalInput")
    bc = nc.dram_tensor("bc", [1, DOUT], F8, kind="ExternalInput")
    out = nc.dram_tensor("out", [SHP, DOUT], F32, kind="ExternalOutput")
    u0s = [nc.dram_tensor(f"u0s{p}", [cfg.piece_rows[p], DMID], F8)
           for p in range(cfg.NPIECE)]
    u0f = [nc.dram_tensor(f"u0f{p}", [cfg.CH[p], DMID], F8,
                          addr_space="Shared") for p in range(cfg.NPIECE)]
    u1s = [nc.dram_tensor(f"u1s{p}", [cfg.piece_rows[p], DMID], F16)
           for p in range(cfg.NPIECE)]
    u1f = [nc.dram_tensor(f"u1f{p}", [cfg.CH[p], DMID], F16,
                          addr_space="Shared") for p in range(cfg.NPIECE)]

    rg = [list(range(NC))]

    def tile_piece(rt):
        p = int(np.searchsorted(pstart_t, rt, side="right") - 1)
        return p, rt - int(pstart_t[p])

    def shard_rows(dram_list, rt):
        p, off = tile_piece(rt)
        return dram_list[p][off * 128:(off + 1) * 128, :]

    with tile.TileContext(nc) as tc:
        with (
            tc.tile_pool(name="const", bufs=1) as cpool,
            tc.tile_pool(name="work", bufs=4) as wpool,
            tc.tile_pool(name="spmm", bufs=2) as gpool,
            tc.tile_pool(name="psum", bufs=2, space="PSUM") as ppool,
        ):
            # ---- constants ----
            ident = cpool.tile([128, 128], F32, tag="ident")
            make_identity(nc, ident[:])
            ident8 = cpool.tile([128, 128], F8, tag="ident8")
            nc.vector.tensor_copy(out=ident8[:], in_=ident[:])
            ident16 = cpool.tile([128, 128], F16, tag="ident16")
            nc.vector.tensor_copy(out=ident16[:], in_=ident[:])
            ones1 = cpool.tile([1, 128], F16, tag="ones1")
            nc.vector.memset(ones1[:], 1.0)
            wd_t = [cpool.tile([128, DMID], F16, tag=f"wd{k}", name=f"wd{k}")
                    for k in range(2)]
            we_t = [cpool.tile([128, DMID], F16, tag=f"we{k}", name=f"we{k}")
                    for k in range(2)]
            wc_t = [cpool.tile([128, DOUT], F16, tag=f"wc{k}", name=f"wc{k}")
                    for k in range(2)]
            for k in range(2):
                nc.sync.dma_start(out=wd_t[k][:], in_=wd[k * 128:(k + 1) * 128, :])
                nc.sync.dma_start(out=we_t[k][:], in_=we[k * 128:(k + 1) * 128, :])
                nc.sync.dma_start(out=wc_t[k][:], in_=wc[k * 128:(k + 1) * 128, :])
            bd_t = cpool.tile([1, DMID], F16, tag="bd")
            be_t = cpool.tile([1, DMID], F8, tag="be")
            bc_t = cpool.tile([1, DOUT], F8, tag="bc")
            nc.gpsimd.dma_start(out=bd_t[:], in_=bd[:])
            nc.sync.dma_start(out=be_t[:], in_=be[:])
            nc.sync.dma_start(out=bc_t[:], in_=bc[:])

            # ---- degree vectors (host-computed) ----
            dinv = cpool.tile([128, NRT], F32, tag="dinv")
            nc.sync.dma_start(out=dinv[:], in_=dinv_d[:])
            dinv2 = cpool.tile([128, NRT], F32, tag="dinv2")
            nc.sync.dma_start(out=dinv2[:], in_=dinv2_d[:])
            sqdrow = cpool.tile([1, NRT * 128], F8, tag="sqdrow")
            nc.sync.dma_start(out=sqdrow[:], in_=sqdrow_d[:])
            nidx1_t = cpool.tile([1, m1["nidx"].shape[1]], I32, tag="nidx1")
            nc.sync.dma_start(out=nidx1_t[:], in_=nidx1[:])
            nidx2_t = cpool.tile([1, m2["nidx"].shape[1]], I32, tag="nidx2")
            nc.sync.dma_start(out=nidx2_t[:], in_=nidx2[:])
            nidx_regs = [nc.gpsimd.alloc_register(f"nidx_reg{c}")
                         for c in range(NCHUNK)]

            # zero both msg buffers once: trailing-trimmed gather slots leave
            # stale bytes which must be finite (they're multiplied by S=0)
            msgmax = max(max(g["tg"] for g in m1["sched"]),
                         2 * max(g["tg"] for g in m2["sched"])) * DMID
            for i in range(3):
                mz = gpool.tile([128, msgmax], F8, tag="msg", name=f"mz{i}",
                                bufs=3)
                nc.vector.memset(mz[:], 0.0)

            # ---- dense layer: u0 = relu(x @ wd + bd) * dinv ----
            # 4-tile batched loads/stores (fewer sequencer issues); batches
            # never cross AllGather piece boundaries.
            for p in range(cfg.NPIECE):
                t0p = int(pstart_t[p])
                for rt0 in range(t0p, t0p + ptiles[p], 4):
                    nt = min(4, t0p + ptiles[p] - rt0)
                    xt = wpool.tile([128, nt, DIN], F16, tag="xt")
                    nc.sync.dma_start(
                        out=xt[:],
                        in_=xtb[rt0 * 128:(rt0 + nt) * 128, :]
                        .rearrange("(t p) d -> p t d", p=128))
                    u0t = wpool.tile([128, nt * DMID], F8, tag="u0t")
                    for j in range(nt):
                        rt = rt0 + j
                        pu = ppool.tile([128, DMID], F32, tag="psu", bufs=3)
                        nc.tensor.matmul(out=pu[:], lhsT=xt[:, j, :128],
                                         rhs=wd_t[0][:],
                                         start=True, stop=False)
                        nc.tensor.matmul(out=pu[:], lhsT=xt[:, j, 128:],
                                         rhs=wd_t[1][:],
                                         start=False, stop=False)
                        nc.tensor.matmul(out=pu[:], lhsT=ones1[:],
                                         rhs=bd_t[:],
                                         start=False, stop=True)
                        nc.scalar.activation(
                            out=u0t[:, j * DMID:(j + 1) * DMID], in_=pu[:],
                            func=mybir.ActivationFunctionType.Relu,
                            scale=dinv[:, rt:rt + 1])
                    off = rt0 - t0p
                    nc.scalar.dma_start(
                        out=u0s[p][off * 128:(off + nt) * 128, :]
                        .rearrange("(t p) d -> p t d", p=128),
                        in_=u0t[:].rearrange("p (t d) -> p t d", d=DMID))
                nc.gpsimd.collective_compute(
                    "AllGather", mybir.AluOpType.bypass, replica_groups=rg,
                    ins=[u0s[p][:]], outs=[u0f[p][:]])

            def conv(m, sv_d, gidx_d, nidx_t, msg_dt, ident_s, u_full, u_selfs,
                     w_tiles, b_tile, out_writer, ag_after):
                pending = []
                ag_ready = []     # pieces whose last window's stage_b is out

                def stage_b(w, ps):
                    s1 = wpool.tile([128, DMID], F32, tag="s1")
                    nc.vector.tensor_copy(out=s1[:], in_=ps[:])
                    s1T = wpool.tile([128, DMID], F16, tag="s1T")
                    for h in range(2):
                        ptr = ppool.tile([128, 128], F32, tag="ptr", bufs=2)
                        nc.tensor.transpose(
                            out=ptr[:], in_=s1[:, h * 128:(h + 1) * 128],
                            identity=ident[:])
                        nc.vector.tensor_copy(
                            out=s1T[:, h * 128:(h + 1) * 128], in_=ptr[:])
                    pu = ppool.tile([128, DMID], F32, tag="psu", bufs=3)
                    nc.tensor.matmul(out=pu[:], lhsT=s1T[:, :128],
                                     rhs=w_tiles[0][:], start=True, stop=False)
                    nc.tensor.matmul(out=pu[:], lhsT=s1T[:, 128:],
                                     rhs=w_tiles[1][:], start=False, stop=False)
                    nc.tensor.matmul(out=pu[:],
                                     lhsT=sqdrow[:, w * 128:(w + 1) * 128],
                                     rhs=b_tile[:], start=False, stop=True)
                    out_writer(w, pu)
                    p, off = tile_piece(w)
                    if ag_after is not None and off + 1 == ptiles[p]:
                        # don't trigger here: the collective's wait would
                        # stall gathers queued behind it on the gpsimd queue
                        # until this window's store lands. Fire it one group
                        # later, when the store has long completed.
                        ag_ready.append(p)

                for g in m["sched"]:
                    ws, tg_g, nblk = g["ws"], g["tg"], g["nblk"]
                    base_msg, c_off = g["base_msg"], g["c_off"]
                    base_blk = g["base_blk"]
                    while ag_ready:
                        ag_after(ag_ready.pop(0))
                    msg = gpool.tile([128, tg_g * DMID], msg_dt, tag="msg",
                                     bufs=3)
                    sst = gpool.tile([128, nblk * 128], msg_dt, tag="sst")
                    gix = gpool.tile([128, tg_g * 8], I16, tag="gix", bufs=4)
                    nc.sync.dma_start(
                        out=gix[:],
                        in_=gidx_d[:, base_msg * 8:(base_msg + tg_g) * 8])
                    ci = 0
                    for c in range(NCHUNK):
                        tgc = (c_off[c + 1] if c + 1 < NCHUNK else tg_g) \
                            - c_off[c]
                        if tgc == 0:
                            continue
                        mo = c_off[c]
                        k = g["call_base"] + ci
                        ci += 1
                        nc.gpsimd.reg_load(nidx_regs[c % 4],
                                           nidx_t[0:1, k:k + 1])
                        nc.gpsimd.dma_gather(
                            msg[:, mo * DMID:(mo + tgc) * DMID]
                                .rearrange("p (t d) -> p t d", d=DMID),
                            u_full[c][:],
                            gix[:, mo * 8:(mo + tgc) * 8],
                            num_idxs=tgc * 128,
                            num_idxs_reg=nidx_regs[c % 4],
                            elem_size=DMID,
                            single_packet=False,
                            queue_num=c % 4,
                        )
                    if msg_dt == F8:
                        nc.sync.dma_start(
                            out=sst[:],
                            in_=sv_d[:, base_blk * 128:(base_blk + nblk) * 128])
                    else:
                        # S stored fp8 in DRAM; DVE-convert to match msg dtype
                        sst8 = gpool.tile([128, nblk * 128], F8, tag="sst8")
                        nc.sync.dma_start(
                            out=sst8[:],
                            in_=sv_d[:, base_blk * 128:(base_blk + nblk) * 128])
                        nc.vector.tensor_copy(out=sst[:], in_=sst8[:])
                    uselfs = []
                    for w in ws:
                        ut = wpool.tile([128, DMID], msg_dt, tag="uself",
                                        bufs=9)
                        nc.scalar.dma_start(out=ut[:],
                                            in_=shard_rows(u_selfs, w))
                        uselfs.append(ut)
                    for wi, w in enumerate(ws):
                        ps = ppool.tile([128, DMID], F32, tag="pss", bufs=3)
                        blocks = g["win_blocks"][wi]
                        for k, (t, b) in enumerate(blocks):
                            nc.tensor.matmul(
                                out=ps[:],
                                lhsT=sst[:, b * 128:(b + 1) * 128],
                                rhs=msg[:, t * DMID:(t + 1) * DMID],
                                start=(k == 0), stop=False)
                        nc.tensor.matmul(out=ps[:], lhsT=ident_s[:],
                                         rhs=uselfs[wi][:],
                                         start=(len(blocks) == 0), stop=True)
                        pending.append((w, ps))
                        if len(pending) >= 2:
                            stage_b(*pending.pop(0))
                for w, ps in pending:
                    stage_b(w, ps)
                while ag_ready:
                    ag_after(ag_ready.pop(0))

            # conv1: u1 = dinv^2*(A@u0)@we + dinv*be   (pre-scaled by dinv)
            def write_u1(w, pu):
                u1t = wpool.tile([128, DMID], F16, tag="u1t")
                nc.scalar.activation(out=u1t[:], in_=pu[:],
                                     func=mybir.ActivationFunctionType.Copy,
                                     scale=dinv2[:, w:w + 1])
                nc.scalar.dma_start(out=shard_rows(u1s, w), in_=u1t[:])

            def ag_u1(p):
                nc.gpsimd.collective_compute(
                    "AllGather", mybir.AluOpType.bypass, replica_groups=rg,
                    ins=[u1s[p][:]], outs=[u1f[p][:]])

            conv(m1, sv1, gidx1, nidx1_t, F8, ident8, u0f, u0s,
                 we_t, be_t, write_u1, ag_u1)

            # conv2: out = dinv*(A@u1)@wc + bc
            def write_out(w, pu):
                ut = wpool.tile([128, DOUT], F32, tag="uout")
                nc.scalar.activation(out=ut[:], in_=pu[:],
                                     func=mybir.ActivationFunctionType.Copy,
                                     scale=dinv[:, w:w + 1])
                nc.scalar.dma_start(out=out[w * 128:(w + 1) * 128, :],
                                    in_=ut[:])

            conv(m2, sv2, gidx2, nidx2_t, F16, ident16, u1f, u1s,
                 wc_t, bc_t, write_out, None)

    nc.compile()
    return nc


def _run(inputs, cfg: Cfg, trace=False):
    x = inputs["x"]
    meta, data = _preprocess(x, inputs["edge_index"], inputs["edge_attr"], cfg)
    nc = _build_program(cfg, meta)

    wcat = np.concatenate([np.asarray(inputs["w_mu"], np.float32),
                           np.asarray(inputs["w_logstd"], np.float32)], axis=1)
    bcat = np.concatenate([np.asarray(inputs["b_mu"], np.float32),
                           np.asarray(inputs["b_logstd"], np.float32)])
    shared = dict(
        wd=np.asarray(inputs["w_dense"], np.float32).astype(NP_F16),
        bd=np.asarray(inputs["b_dense"], np.float32).reshape(1, -1),
        we=np.asarray(inputs["w_enc"], np.float32).astype(NP_F16),
        be=np.asarray(inputs["b_enc"], np.float32).reshape(1, -1)
        .astype(NP_F8),
        wc=wcat.astype(NP_F16), bc=bcat.reshape(1, -1).astype(NP_F8),
    )
    in_maps = []
    for c in range(cfg.NC):
        m = dict(shared)
        m["xtb"] = np.ascontiguousarray(data["xtb"][c])
        m["dinv_d"] = np.ascontiguousarray(data["dinv"][c])
        m["dinv2_d"] = np.ascontiguousarray(data["dinv2"][c])
        m["sqdrow_d"] = np.ascontiguousarray(data["sqdrow"][c])
        m["sv1"] = np.ascontiguousarray(meta["m1"]["sv"][c])
        m["gidx1"] = np.ascontiguousarray(meta["m1"]["gidx"][c])
        m["sv2"] = np.ascontiguousarray(meta["m2"]["sv"][c])
        m["gidx2"] = np.ascontiguousarray(meta["m2"]["gidx"][c])
        m["nidx1"] = np.ascontiguousarray(meta["m1"]["nidx"][c:c + 1])
        m["nidx2"] = np.ascontiguousarray(meta["m2"]["nidx"][c:c + 1])
        in_maps.append(m)

    res = run_bass_kernel_spmd(nc, in_maps, list(range(cfg.NC)), trace=trace)
    SH = cfg.SH
    halves = cfg.DOUT // 2
    mu = np.concatenate([res.results[c]["out"][:SH, :halves]
                         for c in range(cfg.NC)], axis=0)
    ls = np.concatenate([res.results[c]["out"][:SH, halves:]
                         for c in range(cfg.NC)], axis=0)
    return (mu.astype(np.float32), ls.astype(np.float32)), res


def kernel(**inputs):
    cfg = Cfg()
    (mu, ls), _ = _run(inputs, cfg, trace=False)
    return mu, ls


# revision 62
# speedup vs baseline: 1.1463x; 1.1463x over previous
"""GCN encoder (dense+relu -> GCNConv -> {mu, logstd} GCNConv) on 8 Trainium2
NeuronCores.

Strategy (v2):
  - Nodes sharded across 8 cores (12500 rows each, padded to 12544 = 98*128).
  - conv1's message traffic in fp8e4 (gathered rows 256B/edge, scatter
    matrices S, AllGathered table); conv2's messages in f16 with S still fp8
    in DRAM (DVE-converted on load). Error budget: conv1's quantization noise
    is attenuated by conv2's neighborhood averaging; conv2's would hit the
    output directly. PSUM accumulation is f32; dense transforms f16.
  - Edges partitioned by (dest group of GW windows, source chunk); slots
    padded to 128 per (group, chunk) cell only. A tile whose 128 edges span
    several dest windows gets one S block per window it touches (union span
    across cores so the schedule is core-independent). conv1 GW=7, conv2
    GW=2 (f16 tiles are 2x bigger in SBUF).
  - Pad slots carry idx -1 (trailing within each gather call) and the true
    per-core valid count is fed via a runtime register, so the Q7 gather
    ucode skips them entirely (its time is the kernel's hard floor at
    ~2.9ns per real edge, ~1.23ms total).
  - u tables AllGathered in 4 window-aligned pieces (25/25/24/24 row tiles)
    so the next conv's gathers start as soon as their chunk's piece lands.
    AG triggers are deferred one group so their waits never stall gathers
    queued behind them on the gpsimd queue.
  - Degree vectors (dinv, dinv^2, sqrt(deg)) are host-computed inputs; bias
    adds use a rank-1 matmul with the sqrt(deg) row so the per-partition
    output scale folds to the right per-term factors.
  - Dense layer uses host-pretransposed f16 lhsT tiles (no PE transposes),
    4-tile batched loads/stores.
  - Window loop software-pipelined (scatter of w+1 issued before transform
    of w) so PE doesn't stall on the PSUM->SBUF copy round trip; self-loop
    matmul goes last so its DRAM load hides behind the scatter matmuls.
"""
import sys

sys.path.insert(0, "/opt/trn_rl_repo")

import numpy as np
import ml_dtypes

import concourse.bacc as bacc
import concourse.bass as bass
import concourse.mybir as mybir
from concourse import tile
from concourse.bass_utils import run_bass_kernel_spmd
from concourse.masks import make_identity

F32 = mybir.dt.float32
F16 = mybir.dt.float16
F8 = mybir.dt.float8e4
I16 = mybir.dt.int16
NP_F8 = ml_dtypes.float8_e4m3
NP_F16 = np.float16
PAD_TRIM = True
I32 = mybir.dt.int32


class Cfg:
    def __init__(self, N=100000, NC=8, DIN=256, DMID=256, DOUT=256,
                 GW1=7, GW2=2):
        assert N % NC == 0
        self.N, self.NC = N, NC
        self.DIN, self.DMID, self.DOUT = DIN, DMID, DOUT
        self.SH = N // NC                       # real rows per shard
        self.SHP = -(-self.SH // 128) * 128     # padded rows per shard
        self.W = 128                            # dest window size
        self.NW = self.SHP // self.W            # windows (= row tiles)
        self.GW1, self.GW2 = GW1, GW2
        # 4 window-aligned AllGather pieces; piece == gather chunk
        q, r = divmod(self.NW, 4)
        self.piece_tiles = [q + (1 if i < r else 0) for i in range(4)]
        self.piece_rows = [t * 128 for t in self.piece_tiles]
        self.piece_starts = np.concatenate(
            [[0], np.cumsum(self.piece_rows)]).astype(np.int64)
        self.NCHUNK = 4
        self.CH = [NC * r for r in self.piece_rows]   # rows per gather chunk
        assert max(self.CH) <= 32767


def _edge_schedule(core, ldst, cidx, chunk, ew, cfg: Cfg, GW, npdt):
    """Core-independent tile/block schedule + per-core S and idx tables."""
    NC, W, NW, NCHUNK = cfg.NC, cfg.W, cfg.NW, cfg.NCHUNK
    NG = -(-NW // GW)
    win = ldst // W
    grp = win // GW
    wig = win - grp * GW
    dstoff = ldst - win * W

    cell = (core * NG + grp) * NCHUNK + chunk
    order = np.lexsort((cidx, wig, cell))
    cell_s = cell[order]
    n_cells = NC * NG * NCHUNK
    counts = np.bincount(cell, minlength=n_cells).reshape(NC, NG, NCHUNK)

    T_gc = -(-counts.max(axis=0) // 128)            # [NG, NCHUNK]
    c_off = np.zeros((NG, NCHUNK), np.int64)
    c_off[:, 1:] = np.cumsum(T_gc, axis=1)[:, :-1]
    tg = T_gc.sum(axis=1)
    base_g = np.zeros(NG, np.int64)
    base_g[1:] = np.cumsum(tg)[:-1]
    TOT = int(tg.sum())
    tile_base = base_g[:, None] + c_off

    starts = np.zeros(n_cells + 1, np.int64)
    starts[1:] = np.cumsum(counts.reshape(-1))
    rank_s = np.arange(len(cell_s), dtype=np.int64) - starts[cell_s]
    g_s = (cell_s // NCHUNK) % NG
    c_s = cell_s % NCHUNK
    core_s = cell_s // (NG * NCHUNK)
    erow_s = (tile_base[g_s, c_s] + rank_s // 128) * 128 + rank_s % 128

    # pad slots get idx -1: they are trailing within each (g,c) gather call,
    # and the Q7 gather kernel trims trailing negative indices (no descriptor
    # work, no DMA bytes). Their S entries are 0 so stale msg data is killed.
    PAD_IDX = -1 if PAD_TRIM else 0
    IDXRAW = np.full((NC, TOT * 128), PAD_IDX, np.int16)
    IDXRAW[core_s, erow_s] = cidx[order].astype(np.int16)
    idxg = IDXRAW.reshape(NC, TOT * 8, 16).transpose(0, 2, 1)
    IDXG = np.tile(idxg, (1, 8, 1))                 # [NC, 128, TOT*8]

    key4 = cell * GW + wig
    counts4 = np.bincount(key4, minlength=n_cells * GW) \
        .reshape(NC, NG, NCHUNK, GW)
    ends4 = np.cumsum(counts4, axis=3)
    starts4 = ends4 - counts4
    has = counts4 > 0
    T_LO = np.where(has, starts4 // 128, 1 << 30).min(axis=0)
    T_HI = np.where(has, (ends4 - 1) // 128, -1).max(axis=0)
    present = has.any(axis=0)

    blk_start = np.full((NG, NCHUNK, GW), -1, np.int64)
    base_blk = np.zeros(NG, np.int64)
    sched = []
    nblk_total = 0
    nidx_cols = []                     # per gather call: per-core valid count
    for g in range(NG):
        ws = list(range(g * GW, min((g + 1) * GW, NW)))
        base_blk[g] = nblk_total
        call_base = len(nidx_cols)
        for c in range(NCHUNK):
            if T_gc[g, c] > 0:
                nidx_cols.append(counts[:, g, c])
        win_blocks = []
        bi = 0
        for wi in range(len(ws)):
            blocks = []
            for c in range(NCHUNK):
                if not present[g, c, wi]:
                    continue
                blk_start[g, c, wi] = bi
                for t in range(int(T_LO[g, c, wi]), int(T_HI[g, c, wi]) + 1):
                    blocks.append((int(c_off[g, c] + t), bi))
                    bi += 1
            win_blocks.append(blocks)
        nblk_total += bi
        sched.append(dict(ws=ws, base_msg=int(base_g[g]), tg=int(tg[g]),
                          c_off=[int(v) for v in c_off[g]],
                          base_blk=int(base_blk[g]), nblk=bi,
                          call_base=call_base,
                          win_blocks=win_blocks))
    B_TOT = nblk_total
    NIDX = np.stack(nidx_cols, axis=1).astype(np.int32)   # [NC, ncalls]

    t_in_cell = rank_s // 128
    wig_s = wig[order]
    blk_s = (base_blk[g_s] + blk_start[g_s, c_s, wig_s]
             + (t_in_cell - T_LO[g_s, c_s, wig_s]))
    Sv = np.zeros((NC, 128, B_TOT * 128), npdt)
    Sv[core_s, rank_s % 128, blk_s * 128 + dstoff[order]] = \
        ew[order].astype(npdt)

    return dict(TOT=TOT, B_TOT=B_TOT, sched=sched, sv=Sv, gidx=IDXG,
                nidx=NIDX)


def _preprocess(x, edge_index, edge_attr, cfg: Cfg):
    NC, SH, SHP = cfg.NC, cfg.SH, cfg.SHP

    src = np.asarray(edge_index[0], dtype=np.int64)
    dst = np.asarray(edge_index[1], dtype=np.int64)
    ew = np.asarray(edge_attr, dtype=np.float32)
    core = dst // SH
    ldst = dst - core * SH
    s_shard = src // SH
    s_loc = src - s_shard * SH
    s_piece = np.searchsorted(cfg.piece_starts, s_loc, side="right") - 1
    prow = np.asarray(cfg.piece_rows, np.int64)
    cidx = s_shard * prow[s_piece] + (s_loc - cfg.piece_starts[s_piece])

    m1 = _edge_schedule(core, ldst, cidx, s_piece, ew, cfg, cfg.GW1, NP_F8)
    m2 = _edge_schedule(core, ldst, cidx, s_piece, ew, cfg, cfg.GW2, NP_F8)

    # degree (with self-loop weight 1) computed on host
    deg = np.bincount(core * SHP + ldst, weights=ew.astype(np.float64),
                      minlength=NC * SHP).reshape(NC, SHP) \
        .astype(np.float32) + 1.0
    dinv2 = 1.0 / deg
    dinv = np.sqrt(dinv2)
    sqd = np.sqrt(deg)
    NW = cfg.NW
    # [NC, 128, NW] layout: [c, p, rt] = value at row rt*128+p
    dinv_t = dinv.reshape(NC, NW, 128).transpose(0, 2, 1).copy()
    dinv2_t = dinv2.reshape(NC, NW, 128).transpose(0, 2, 1).copy()
    sqdrow = sqd.reshape(NC, 1, SHP).astype(NP_F8)

    # dense lhsT tiles, host-transposed, f16:
    # xtb[c, rt*128+p, h*128+j] = x[c*SH + rt*128 + j, h*128 + p]
    xs = np.zeros((NC, SHP, cfg.DIN), np.float32)
    xs[:, :SH, :] = np.asarray(x, np.float32).reshape(NC, SH, cfg.DIN)
    xtb = xs.reshape(NC, NW, 128, cfg.DIN).transpose(0, 1, 3, 2) \
        .reshape(NC, NW, 2, 128, 128).transpose(0, 1, 3, 2, 4) \
        .reshape(NC, NW * 128, cfg.DIN).astype(NP_F16)

    meta = dict(m1=m1, m2=m2)
    data = dict(xtb=xtb, dinv=dinv_t, dinv2=dinv2_t, sqdrow=sqdrow)
    return meta, data


def _build_program(cfg: Cfg, meta):
    NC, SHP, W, NW = cfg.NC, cfg.SHP, cfg.W, cfg.NW
    DIN, DMID, DOUT = cfg.DIN, cfg.DMID, cfg.DOUT
    NCHUNK = cfg.NCHUNK
    m1, m2 = meta["m1"], meta["m2"]
    NRT = NW
    ptiles = cfg.piece_tiles
    pstart_t = np.concatenate([[0], np.cumsum(ptiles)])

    nc = bacc.Bacc("TRN2", target_bir_lowering=False, debug=False,
                   num_devices=NC, num_swdge_queues=4)

    xtb = nc.dram_tensor("xtb", [SHP, DIN], F16, kind="ExternalInput")
    dinv_d = nc.dram_tensor("dinv_d", [128, NRT], F32, kind="ExternalInput")
    dinv2_d = nc.dram_tensor("dinv2_d", [128, NRT], F32, kind="ExternalInput")
    sqdrow_d = nc.dram_tensor("sqdrow_d", [1, SHP], F8, kind="ExternalInput")
    sv1 = nc.dram_tensor("sv1", [128, m1["B_TOT"] * 128], F8,
                         kind="ExternalInput")
    gidx1 = nc.dram_tensor("gidx1", [128, m1["TOT"] * 8], I16,
                           kind="ExternalInput")
    sv2 = nc.dram_tensor("sv2", [128, m2["B_TOT"] * 128], F8,
                         kind="ExternalInput")
    gidx2 = nc.dram_tensor("gidx2", [128, m2["TOT"] * 8], I16,
                           kind="ExternalInput")
    nidx1 = nc.dram_tensor("nidx1", [1, m1["nidx"].shape[1]], I32,
                           kind="ExternalInput")
    nidx2 = nc.dram_tensor("nidx2", [1, m2["nidx"].shape[1]], I32,
                           kind="ExternalInput")
    wd = nc.dram_tensor("wd", [DIN, DMID], F16, kind="ExternalInput")
    bd = nc.dram_tensor("bd", [1, DMID], F32, kind="ExternalInput")
    we = nc.dram_tensor("we", [DMID, DMID], F16, kind="ExternalInput")
    be = nc.dram_tensor("be", [1, DMID], F8, kind="ExternalInput")
    wc = nc.dram_tensor("wc", [DMID, DOUT], F16, kind="ExternalInput")
    bc = nc.dram_tensor("bc", [1, DOUT], F8, kind="ExternalInput")
    out = nc.dram_tensor("out", [SHP, DOUT], F32, kind="ExternalOutput")
    u0s = [nc.dram_tensor(f"u0s{p}", [cfg.piece_rows[p], DMID], F8)
           for p in range(4)]
    u0f = [nc.dram_tensor(f"u0f{p}", [cfg.CH[p], DMID], F8,
                          addr_space="Shared") for p in range(4)]
    u1s = [nc.dram_tensor(f"u1s{p}", [cfg.piece_rows[p], DMID], F16)
           for p in range(4)]
    u1f = [nc.dram_tensor(f"u1f{p}", [cfg.CH[p], DMID], F16,
                          addr_space="Shared") for p in range(4)]

    rg = [list(range(NC))]

    def tile_piece(rt):
        p = int(np.searchsorted(pstart_t, rt, side="right") - 1)
        return p, rt - int(pstart_t[p])

    def shard_rows(dram_list, rt):
        p, off = tile_piece(rt)
        return dram_list[p][off * 128:(off + 1) * 128, :]

    with tile.TileContext(nc) as tc:
        with (
            tc.tile_pool(name="const", bufs=1) as cpool,
            tc.tile_pool(name="work", bufs=4) as wpool,
            tc.tile_pool(name="spmm", bufs=2) as gpool,
            tc.tile_pool(name="psum", bufs=2, space="PSUM") as ppool,
        ):
            # ---- constants ----
            ident = cpool.tile([128, 128], F32, tag="ident")
            make_identity(nc, ident[:])
            ident8 = cpool.tile([128, 128], F8, tag="ident8")
            nc.vector.tensor_copy(out=ident8[:], in_=ident[:])
            ident16 = cpool.tile([128, 128], F16, tag="ident16")
            nc.vector.tensor_copy(out=ident16[:], in_=ident[:])
            ones1 = cpool.tile([1, 128], F16, tag="ones1")
            nc.vector.memset(ones1[:], 1.0)
            wd_t = [cpool.tile([128, DMID], F16, tag=f"wd{k}", name=f"wd{k}")
                    for k in range(2)]
            we_t = [cpool.tile([128, DMID], F16, tag=f"we{k}", name=f"we{k}")
                    for k in range(2)]
            wc_t = [cpool.tile([128, DOUT], F16, tag=f"wc{k}", name=f"wc{k}")
                    for k in range(2)]
            for k in range(2):
                nc.sync.dma_start(out=wd_t[k][:], in_=wd[k * 128:(k + 1) * 128, :])
                nc.sync.dma_start(out=we_t[k][:], in_=we[k * 128:(k + 1) * 128, :])
                nc.sync.dma_start(out=wc_t[k][:], in_=wc[k * 128:(k + 1) * 128, :])
            bd_t = cpool.tile([1, DMID], F16, tag="bd")
            be_t = cpool.tile([1, DMID], F8, tag="be")
            bc_t = cpool.tile([1, DOUT], F8, tag="bc")
            nc.gpsimd.dma_start(out=bd_t[:], in_=bd[:])
            nc.sync.dma_start(out=be_t[:], in_=be[:])
            nc.sync.dma_start(out=bc_t[:], in_=bc[:])

            # ---- degree vectors (host-computed) ----
            dinv = cpool.tile([128, NRT], F32, tag="dinv")
            nc.sync.dma_start(out=dinv[:], in_=dinv_d[:])
            dinv2 = cpool.tile([128, NRT], F32, tag="dinv2")
            nc.sync.dma_start(out=dinv2[:], in_=dinv2_d[:])
            sqdrow = cpool.tile([1, NRT * 128], F8, tag="sqdrow")
            nc.sync.dma_start(out=sqdrow[:], in_=sqdrow_d[:])
            nidx1_t = cpool.tile([1, m1["nidx"].shape[1]], I32, tag="nidx1")
            nc.sync.dma_start(out=nidx1_t[:], in_=nidx1[:])
            nidx2_t = cpool.tile([1, m2["nidx"].shape[1]], I32, tag="nidx2")
            nc.sync.dma_start(out=nidx2_t[:], in_=nidx2[:])
            nidx_regs = [nc.gpsimd.alloc_register(f"nidx_reg{c}")
                         for c in range(NCHUNK)]

            # zero both msg buffers once: trailing-trimmed gather slots leave
            # stale bytes which must be finite (they're multiplied by S=0)
            msgmax = max(max(g["tg"] for g in m1["sched"]),
                         2 * max(g["tg"] for g in m2["sched"])) * DMID
            for i in range(3):
                mz = gpool.tile([128, msgmax], F8, tag="msg", name=f"mz{i}",
                                bufs=3)
                nc.vector.memset(mz[:], 0.0)

            # ---- dense layer: u0 = relu(x @ wd + bd) * dinv ----
            # 4-tile batched loads/stores (fewer sequencer issues); batches
            # never cross AllGather piece boundaries.
            for p in range(4):
                t0p = int(pstart_t[p])
                for rt0 in range(t0p, t0p + ptiles[p], 4):
                    nt = min(4, t0p + ptiles[p] - rt0)
                    xt = wpool.tile([128, nt, DIN], F16, tag="xt")
                    nc.sync.dma_start(
                        out=xt[:],
                        in_=xtb[rt0 * 128:(rt0 + nt) * 128, :]
                        .rearrange("(t p) d -> p t d", p=128))
                    u0t = wpool.tile([128, nt * DMID], F8, tag="u0t")
                    for j in range(nt):
                        rt = rt0 + j
                        pu = ppool.tile([128, DMID], F32, tag="psu", bufs=3)
                        nc.tensor.matmul(out=pu[:], lhsT=xt[:, j, :128],
                                         rhs=wd_t[0][:],
                                         start=True, stop=False)
                        nc.tensor.matmul(out=pu[:], lhsT=xt[:, j, 128:],
                                         rhs=wd_t[1][:],
                                         start=False, stop=False)
                        nc.tensor.matmul(out=pu[:], lhsT=ones1[:],
                                         rhs=bd_t[:],
                                         start=False, stop=True)
                        nc.scalar.activation(
                            out=u0t[:, j * DMID:(j + 1) * DMID], in_=pu[:],
                            func=mybir.ActivationFunctionType.Relu,
                            scale=dinv[:, rt:rt + 1])
                    off = rt0 - t0p
                    nc.scalar.dma_start(
                        out=u0s[p][off * 128:(off + nt) * 128, :]
                        .rearrange("(t p) d -> p t d", p=128),
                        in_=u0t[:].rearrange("p (t d) -> p t d", d=DMID))
                nc.gpsimd.collective_compute(
                    "AllGather", mybir.AluOpType.bypass, replica_groups=rg,
                    ins=[u0s[p][:]], outs=[u0f[p][:]])

            def conv(m, sv_d, gidx_d, nidx_t, msg_dt, ident_s, u_full, u_selfs,
                     w_tiles, b_tile, out_writer, ag_after):
                pending = []
                ag_ready = []     # pieces whose last window's stage_b is out

                def stage_b(w, ps):
                    s1 = wpool.tile([128, DMID], F32, tag="s1")
                    nc.vector.tensor_copy(out=s1[:], in_=ps[:])
                    s1T = wpool.tile([128, DMID], F16, tag="s1T")
                    for h in range(2):
                        ptr = ppool.tile([128, 128], F32, tag="ptr", bufs=2)
                        nc.tensor.transpose(
                            out=ptr[:], in_=s1[:, h * 128:(h + 1) * 128],
                            identity=ident[:])
                        nc.vector.tensor_copy(
                            out=s1T[:, h * 128:(h + 1) * 128], in_=ptr[:])
                    pu = ppool.tile([128, DMID], F32, tag="psu", bufs=3)
                    nc.tensor.matmul(out=pu[:], lhsT=s1T[:, :128],
                                     rhs=w_tiles[0][:], start=True, stop=False)
                    nc.tensor.matmul(out=pu[:], lhsT=s1T[:, 128:],
                                     rhs=w_tiles[1][:], start=False, stop=False)
                    nc.tensor.matmul(out=pu[:],
                                     lhsT=sqdrow[:, w * 128:(w + 1) * 128],
                                     rhs=b_tile[:], start=False, stop=True)
                    out_writer(w, pu)
                    p, off = tile_piece(w)
                    if ag_after is not None and off + 1 == ptiles[p]:
                        # don't trigger here: the collective's wait would
                        # stall gathers queued behind it on the gpsimd queue
                        # until this window's store lands. Fire it one group
                        # later, when the store has long completed.
                        ag_ready.append(p)

                for g in m["sched"]:
                    ws, tg_g, nblk = g["ws"], g["tg"], g["nblk"]
                    base_msg, c_off = g["base_msg"], g["c_off"]
                    base_blk = g["base_blk"]
                    while ag_ready:
                        ag_after(ag_ready.pop(0))
                    msg = gpool.tile([128, tg_g * DMID], msg_dt, tag="msg",
                                     bufs=3)
                    sst = gpool.tile([128, nblk * 128], msg_dt, tag="sst")
                    gix = gpool.tile([128, tg_g * 8], I16, tag="gix", bufs=4)
                    nc.sync.dma_start(
                        out=gix[:],
                        in_=gidx_d[:, base_msg * 8:(base_msg + tg_g) * 8])
                    ci = 0
                    for c in range(NCHUNK):
                        tgc = (c_off[c + 1] if c + 1 < NCHUNK else tg_g) \
                            - c_off[c]
                        if tgc == 0:
                            continue
                        mo = c_off[c]
                        k = g["call_base"] + ci
                        ci += 1
                        nc.gpsimd.reg_load(nidx_regs[c], nidx_t[0:1, k:k + 1])
                        nc.gpsimd.dma_gather(
                            msg[:, mo * DMID:(mo + tgc) * DMID]
                                .rearrange("p (t d) -> p t d", d=DMID),
                            u_full[c][:],
                            gix[:, mo * 8:(mo + tgc) * 8],
                            num_idxs=tgc * 128,
                            num_idxs_reg=nidx_regs[c],
                            elem_size=DMID,
                            single_packet=False,
                            queue_num=c,
                        )
                    if msg_dt == F8:
                        nc.sync.dma_start(
                            out=sst[:],
                            in_=sv_d[:, base_blk * 128:(base_blk + nblk) * 128])
                    else:
                        # S stored fp8 in DRAM; DVE-convert to match msg dtype
                        sst8 = gpool.tile([128, nblk * 128], F8, tag="sst8")
                        nc.sync.dma_start(
                            out=sst8[:],
                            in_=sv_d[:, base_blk * 128:(base_blk + nblk) * 128])
                        nc.vector.tensor_copy(out=sst[:], in_=sst8[:])
                    uselfs = []
                    for w in ws:
                        ut = wpool.tile([128, DMID], msg_dt, tag="uself",
                                        bufs=9)
                        nc.scalar.dma_start(out=ut[:],
                                            in_=shard_rows(u_selfs, w))
                        uselfs.append(ut)
                    for wi, w in enumerate(ws):
                        ps = ppool.tile([128, DMID], F32, tag="pss", bufs=3)
                        blocks = g["win_blocks"][wi]
                        for k, (t, b) in enumerate(blocks):
                            nc.tensor.matmul(
                                out=ps[:],
                                lhsT=sst[:, b * 128:(b + 1) * 128],
                                rhs=msg[:, t * DMID:(t + 1) * DMID],
                                start=(k == 0), stop=False)
                        nc.tensor.matmul(out=ps[:], lhsT=ident_s[:],
                                         rhs=uselfs[wi][:],
                                         start=(len(blocks) == 0), stop=True)
                        pending.append((w, ps))
                        if len(pending) >= 2:
                            stage_b(*pending.pop(0))
                for w, ps in pending:
                    stage_b(w, ps)
                while ag_ready:
                    ag_after(ag_ready.pop(0))

            # conv1: u1 = dinv^2*(A@u0)@we + dinv*be   (pre-scaled by dinv)
            def write_u1(w, pu):
                u1t = wpool.tile([128, DMID], F16, tag="u1t")
                nc.scalar.activation(out=u1t[:], in_=pu[:],
                                     func=mybir.ActivationFunctionType.Copy,
                                     scale=dinv2[:, w:w + 1])
                nc.scalar.dma_start(out=shard_rows(u1s, w), in_=u1t[:])

            def ag_u1(p):
                nc.gpsimd.collective_compute(
                    "AllGather", mybir.AluOpType.bypass, replica_groups=rg,
                    ins=[u1s[p][:]], outs=[u1f[p][:]])

            conv(m1, sv1, gidx1, nidx1_t, F8, ident8, u0f, u0s,
                 we_t, be_t, write_u1, ag_u1)

            # conv2: out = dinv*(A@u1)@wc + bc
            def write_out(w, pu):
                ut = wpool.tile([128, DOUT], F32, tag="uout")
                nc.scalar.activation(out=ut[:], in_=pu[:],
                                     func=mybir.ActivationFunctionType.Copy,
                                     scale=dinv[:, w:w + 1])
                nc.scalar.dma_start(out=out[w * 128:(w + 1) * 128, :],
                                    in_=ut[:])

            conv(m2, sv2, gidx2, nidx2_t, F16, ident16, u1f, u1s,
                 wc_t, bc_t, write_out, None)

    nc.compile()
    return nc


def _run(inputs, cfg: Cfg, trace=False):
    x = inputs["x"]
    meta, data = _preprocess(x, inputs["edge_index"], inputs["edge_attr"], cfg)
    nc = _build_program(cfg, meta)

    wcat = np.concatenate([np.asarray(inputs["w_mu"], np.float32),
                           np.asarray(inputs["w_logstd"], np.float32)], axis=1)
    bcat = np.concatenate([np.asarray(inputs["b_mu"], np.float32),
                           np.asarray(inputs["b_logstd"], np.float32)])
    shared = dict(
        wd=np.asarray(inputs["w_dense"], np.float32).astype(NP_F16),
        bd=np.asarray(inputs["b_dense"], np.float32).reshape(1, -1),
        we=np.asarray(inputs["w_enc"], np.float32).astype(NP_F16),
        be=np.asarray(inputs["b_enc"], np.float32).reshape(1, -1)
        .astype(NP_F8),
        wc=wcat.astype(NP_F16), bc=bcat.reshape(1, -1).astype(NP_F8),
    )
    in_maps = []
    for c in range(cfg.NC):
        m = dict(shared)
        m["xtb"] = np.ascontiguousarray(data["xtb"][c])
        m["dinv_d"] = np.ascontiguousarray(data["dinv"][c])
        m["dinv2_d"] = np.ascontiguousarray(data["dinv2"][c])
        m["sqdrow_d"] = np.ascontiguousarray(data["sqdrow"][c])
        m["sv1"] = np.ascontiguousarray(meta["m1"]["sv"][c])
        m["gidx1"] = np.ascontiguousarray(meta["m1"]["gidx"][c])
        m["sv2"] = np.ascontiguousarray(meta["m2"]["sv"][c])
        m["gidx2"] = np.ascontiguousarray(meta["m2"]["gidx"][c])
        m["nidx1"] = np.ascontiguousarray(meta["m1"]["nidx"][c:c + 1])
        m["nidx2"] = np.ascontiguousarray(meta["m2"]["nidx"][c:c + 1])
        in_maps.append(m)

    res = run_bass_kernel_spmd(nc, in_maps, list(range(cfg.NC)), trace=trace)
    SH = cfg.SH
    halves = cfg.DOUT // 2
    mu = np.concatenate([res.results[c]["out"][:SH, :halves]
                         for c in range(cfg.NC)], axis=0)
    ls = np.concatenate([res.results[c]["out"][:SH, halves:]
                         for c in range(cfg.NC)], axis=0)
    return (mu.astype(np.float32), ls.astype(np.float32)), res


def kernel(**inputs):
    cfg = Cfg()
    (mu, ls), _ = _run(inputs, cfg, trace=False)
    return mu, ls


# revision 64
# speedup vs baseline: 1.3126x; 1.1451x over previous
"""GCN encoder (dense+relu -> GCNConv -> {mu, logstd} GCNConv) on 8 Trainium2
NeuronCores.

Strategy (v2):
  - Nodes sharded across 8 cores (12500 rows each, padded to 12544 = 98*128).
  - conv1's message traffic in fp8e4 (gathered rows 256B/edge, scatter
    matrices S, AllGathered table); conv2's messages in f16 with S still fp8
    in DRAM (DVE-converted on load). Error budget: conv1's quantization noise
    is attenuated by conv2's neighborhood averaging; conv2's would hit the
    output directly. PSUM accumulation is f32; dense transforms f16.
  - Edges partitioned by (dest group of GW windows, source chunk); slots
    padded to 128 per (group, chunk) cell only. A tile whose 128 edges span
    several dest windows gets one S block per window it touches (union span
    across cores so the schedule is core-independent). conv1 GW=7, conv2
    GW=2 (f16 tiles are 2x bigger in SBUF).
  - Pad slots carry idx -1 (trailing within each gather call) and the true
    per-core valid count is fed via a runtime register, so the Q7 gather
    ucode skips them entirely (its time is the kernel's hard floor at
    ~2.9ns per real edge, ~1.23ms total).
  - u tables AllGathered in 4 window-aligned pieces (25/25/24/24 row tiles)
    so the next conv's gathers start as soon as their chunk's piece lands.
    AG triggers are deferred one group so their waits never stall gathers
    queued behind them on the gpsimd queue.
  - Degree vectors (dinv, dinv^2, sqrt(deg)) are host-computed inputs; bias
    adds use a rank-1 matmul with the sqrt(deg) row so the per-partition
    output scale folds to the right per-term factors.
  - Dense layer uses host-pretransposed f16 lhsT tiles (no PE transposes),
    4-tile batched loads/stores.
  - Window loop software-pipelined (scatter of w+1 issued before transform
    of w) so PE doesn't stall on the PSUM->SBUF copy round trip; self-loop
    matmul goes last so its DRAM load hides behind the scatter matmuls.
"""
import sys

sys.path.insert(0, "/opt/trn_rl_repo")

import numpy as np
import ml_dtypes

import concourse.bacc as bacc
import concourse.bass as bass
import concourse.mybir as mybir
from concourse import tile
from concourse.bass_utils import run_bass_kernel_spmd
from concourse.masks import make_identity

F32 = mybir.dt.float32
F16 = mybir.dt.float16
F8 = mybir.dt.float8e4
I16 = mybir.dt.int16
NP_F8 = ml_dtypes.float8_e4m3
NP_F16 = np.float16
PAD_TRIM = True
I32 = mybir.dt.int32


class Cfg:
    def __init__(self, N=100000, NC=8, DIN=256, DMID=256, DOUT=256,
                 GW1=7, GW2=2):
        assert N % NC == 0
        self.N, self.NC = N, NC
        self.DIN, self.DMID, self.DOUT = DIN, DMID, DOUT
        self.SH = N // NC                       # real rows per shard
        self.SHP = -(-self.SH // 128) * 128     # padded rows per shard
        self.W = 128                            # dest window size
        self.NW = self.SHP // self.W            # windows (= row tiles)
        self.GW1, self.GW2 = GW1, GW2
        # 4 window-aligned AllGather pieces; piece == gather chunk
        q, r = divmod(self.NW, 4)
        self.piece_tiles = [q + (1 if i < r else 0) for i in range(4)]
        self.piece_rows = [t * 128 for t in self.piece_tiles]
        self.piece_starts = np.concatenate(
            [[0], np.cumsum(self.piece_rows)]).astype(np.int64)
        self.NCHUNK = 4
        self.CH = [NC * r for r in self.piece_rows]   # rows per gather chunk
        assert max(self.CH) <= 32767


def _edge_schedule(core, ldst, cidx, chunk, ew, cfg: Cfg, GW, npdt):
    """Core-independent tile/block schedule + per-core S and idx tables."""
    NC, W, NW, NCHUNK = cfg.NC, cfg.W, cfg.NW, cfg.NCHUNK
    NG = -(-NW // GW)
    win = ldst // W
    grp = win // GW
    wig = win - grp * GW
    dstoff = ldst - win * W

    cell = (core * NG + grp) * NCHUNK + chunk
    order = np.lexsort((cidx, wig, cell))
    cell_s = cell[order]
    n_cells = NC * NG * NCHUNK
    counts = np.bincount(cell, minlength=n_cells).reshape(NC, NG, NCHUNK)

    T_gc = -(-counts.max(axis=0) // 128)            # [NG, NCHUNK]
    c_off = np.zeros((NG, NCHUNK), np.int64)
    c_off[:, 1:] = np.cumsum(T_gc, axis=1)[:, :-1]
    tg = T_gc.sum(axis=1)
    base_g = np.zeros(NG, np.int64)
    base_g[1:] = np.cumsum(tg)[:-1]
    TOT = int(tg.sum())
    tile_base = base_g[:, None] + c_off

    starts = np.zeros(n_cells + 1, np.int64)
    starts[1:] = np.cumsum(counts.reshape(-1))
    rank_s = np.arange(len(cell_s), dtype=np.int64) - starts[cell_s]
    g_s = (cell_s // NCHUNK) % NG
    c_s = cell_s % NCHUNK
    core_s = cell_s // (NG * NCHUNK)
    erow_s = (tile_base[g_s, c_s] + rank_s // 128) * 128 + rank_s % 128

    # pad slots get idx -1: they are trailing within each (g,c) gather call,
    # and the Q7 gather kernel trims trailing negative indices (no descriptor
    # work, no DMA bytes). Their S entries are 0 so stale msg data is killed.
    PAD_IDX = -1 if PAD_TRIM else 0
    IDXRAW = np.full((NC, TOT * 128), PAD_IDX, np.int16)
    IDXRAW[core_s, erow_s] = cidx[order].astype(np.int16)
    idxg = IDXRAW.reshape(NC, TOT * 8, 16).transpose(0, 2, 1)
    IDXG = np.tile(idxg, (1, 8, 1))                 # [NC, 128, TOT*8]

    key4 = cell * GW + wig
    counts4 = np.bincount(key4, minlength=n_cells * GW) \
        .reshape(NC, NG, NCHUNK, GW)
    ends4 = np.cumsum(counts4, axis=3)
    starts4 = ends4 - counts4
    has = counts4 > 0
    T_LO = np.where(has, starts4 // 128, 1 << 30).min(axis=0)
    T_HI = np.where(has, (ends4 - 1) // 128, -1).max(axis=0)
    present = has.any(axis=0)

    blk_start = np.full((NG, NCHUNK, GW), -1, np.int64)
    base_blk = np.zeros(NG, np.int64)
    sched = []
    nblk_total = 0
    nidx_cols = []                     # per gather call: per-core valid count
    for g in range(NG):
        ws = list(range(g * GW, min((g + 1) * GW, NW)))
        base_blk[g] = nblk_total
        call_base = len(nidx_cols)
        for c in range(NCHUNK):
            if T_gc[g, c] > 0:
                nidx_cols.append(counts[:, g, c])
        win_blocks = []
        bi = 0
        for wi in range(len(ws)):
            blocks = []
            for c in range(NCHUNK):
                if not present[g, c, wi]:
                    continue
                blk_start[g, c, wi] = bi
                for t in range(int(T_LO[g, c, wi]), int(T_HI[g, c, wi]) + 1):
                    blocks.append((int(c_off[g, c] + t), bi))
                    bi += 1
            win_blocks.append(blocks)
        nblk_total += bi
        sched.append(dict(ws=ws, base_msg=int(base_g[g]), tg=int(tg[g]),
                          c_off=[int(v) for v in c_off[g]],
                          base_blk=int(base_blk[g]), nblk=bi,
                          call_base=call_base,
                          win_blocks=win_blocks))
    B_TOT = nblk_total
    NIDX = np.stack(nidx_cols, axis=1).astype(np.int32)   # [NC, ncalls]

    t_in_cell = rank_s // 128
    wig_s = wig[order]
    blk_s = (base_blk[g_s] + blk_start[g_s, c_s, wig_s]
             + (t_in_cell - T_LO[g_s, c_s, wig_s]))
    Sv = np.zeros((NC, 128, B_TOT * 128), npdt)
    Sv[core_s, rank_s % 128, blk_s * 128 + dstoff[order]] = \
        ew[order].astype(npdt)

    return dict(TOT=TOT, B_TOT=B_TOT, sched=sched, sv=Sv, gidx=IDXG,
                nidx=NIDX)


def _preprocess(x, edge_index, edge_attr, cfg: Cfg):
    NC, SH, SHP = cfg.NC, cfg.SH, cfg.SHP

    src = np.asarray(edge_index[0], dtype=np.int64)
    dst = np.asarray(edge_index[1], dtype=np.int64)
    ew = np.asarray(edge_attr, dtype=np.float32)
    core = dst // SH
    ldst = dst - core * SH
    s_shard = src // SH
    s_loc = src - s_shard * SH
    s_piece = np.searchsorted(cfg.piece_starts, s_loc, side="right") - 1
    prow = np.asarray(cfg.piece_rows, np.int64)
    cidx = s_shard * prow[s_piece] + (s_loc - cfg.piece_starts[s_piece])

    m1 = _edge_schedule(core, ldst, cidx, s_piece, ew, cfg, cfg.GW1, NP_F8)
    m2 = _edge_schedule(core, ldst, cidx, s_piece, ew, cfg, cfg.GW2, NP_F8)

    # degree (with self-loop weight 1) computed on host
    deg = np.bincount(core * SHP + ldst, weights=ew.astype(np.float64),
                      minlength=NC * SHP).reshape(NC, SHP) \
        .astype(np.float32) + 1.0
    dinv2 = 1.0 / deg
    dinv = np.sqrt(dinv2)
    sqd = np.sqrt(deg)
    NW = cfg.NW
    # [NC, 128, NW] layout: [c, p, rt] = value at row rt*128+p
    dinv_t = dinv.reshape(NC, NW, 128).transpose(0, 2, 1).copy()
    dinv2_t = dinv2.reshape(NC, NW, 128).transpose(0, 2, 1).copy()
    sqdrow = sqd.reshape(NC, 1, SHP).astype(NP_F8)

    # dense lhsT tiles, host-transposed, f16:
    # xtb[c, rt*128+p, h*128+j] = x[c*SH + rt*128 + j, h*128 + p]
    xs = np.zeros((NC, SHP, cfg.DIN), np.float32)
    xs[:, :SH, :] = np.asarray(x, np.float32).reshape(NC, SH, cfg.DIN)
    xtb = xs.reshape(NC, NW, 128, cfg.DIN).transpose(0, 1, 3, 2) \
        .reshape(NC, NW, 2, 128, 128).transpose(0, 1, 3, 2, 4) \
        .reshape(NC, NW * 128, cfg.DIN).astype(NP_F16)

    meta = dict(m1=m1, m2=m2)
    data = dict(xtb=xtb, dinv=dinv_t, dinv2=dinv2_t, sqdrow=sqdrow)
    return meta, data


def _build_program(cfg: Cfg, meta):
    NC, SHP, W, NW = cfg.NC, cfg.SHP, cfg.W, cfg.NW
    DIN, DMID, DOUT = cfg.DIN, cfg.DMID, cfg.DOUT
    NCHUNK = cfg.NCHUNK
    m1, m2 = meta["m1"], meta["m2"]
    NRT = NW
    ptiles = cfg.piece_tiles
    pstart_t = np.concatenate([[0], np.cumsum(ptiles)])

    nc = bacc.Bacc("TRN2", target_bir_lowering=False, debug=False,
                   num_devices=NC, num_swdge_queues=4)

    xtb = nc.dram_tensor("xtb", [SHP, DIN], F16, kind="ExternalInput")
    dinv_d = nc.dram_tensor("dinv_d", [128, NRT], F32, kind="ExternalInput")
    dinv2_d = nc.dram_tensor("dinv2_d", [128, NRT], F32, kind="ExternalInput")
    sqdrow_d = nc.dram_tensor("sqdrow_d", [1, SHP], F8, kind="ExternalInput")
    sv1 = nc.dram_tensor("sv1", [128, m1["B_TOT"] * 128], F8,
                         kind="ExternalInput")
    gidx1 = nc.dram_tensor("gidx1", [128, m1["TOT"] * 8], I16,
                           kind="ExternalInput")
    sv2 = nc.dram_tensor("sv2", [128, m2["B_TOT"] * 128], F8,
                         kind="ExternalInput")
    gidx2 = nc.dram_tensor("gidx2", [128, m2["TOT"] * 8], I16,
                           kind="ExternalInput")
    nidx1 = nc.dram_tensor("nidx1", [1, m1["nidx"].shape[1]], I32,
                           kind="ExternalInput")
    nidx2 = nc.dram_tensor("nidx2", [1, m2["nidx"].shape[1]], I32,
                           kind="ExternalInput")
    wd = nc.dram_tensor("wd", [DIN, DMID], F16, kind="ExternalInput")
    bd = nc.dram_tensor("bd", [1, DMID], F32, kind="ExternalInput")
    we = nc.dram_tensor("we", [DMID, DMID], F16, kind="ExternalInput")
    be = nc.dram_tensor("be", [1, DMID], F8, kind="ExternalInput")
    wc = nc.dram_tensor("wc", [DMID, DOUT], F16, kind="ExternalInput")
    bc = nc.dram_tensor("bc", [1, DOUT], F8, kind="ExternalInput")
    out = nc.dram_tensor("out", [SHP, DOUT], F32, kind="ExternalOutput")
    u0s = [nc.dram_tensor(f"u0s{p}", [cfg.piece_rows[p], DMID], F8)
           for p in range(4)]
    u0f = [nc.dram_tensor(f"u0f{p}", [cfg.CH[p], DMID], F8,
                          addr_space="Shared") for p in range(4)]
    u1s = [nc.dram_tensor(f"u1s{p}", [cfg.piece_rows[p], DMID], F8)
           for p in range(4)]
    u1f = [nc.dram_tensor(f"u1f{p}", [cfg.CH[p], DMID], F8,
                          addr_space="Shared") for p in range(4)]

    rg = [list(range(NC))]

    def tile_piece(rt):
        p = int(np.searchsorted(pstart_t, rt, side="right") - 1)
        return p, rt - int(pstart_t[p])

    def shard_rows(dram_list, rt):
        p, off = tile_piece(rt)
        return dram_list[p][off * 128:(off + 1) * 128, :]

    with tile.TileContext(nc) as tc:
        with (
            tc.tile_pool(name="const", bufs=1) as cpool,
            tc.tile_pool(name="work", bufs=4) as wpool,
            tc.tile_pool(name="spmm", bufs=2) as gpool,
            tc.tile_pool(name="psum", bufs=2, space="PSUM") as ppool,
        ):
            # ---- constants ----
            ident = cpool.tile([128, 128], F32, tag="ident")
            make_identity(nc, ident[:])
            ident8 = cpool.tile([128, 128], F8, tag="ident8")
            nc.vector.tensor_copy(out=ident8[:], in_=ident[:])
            ident16 = cpool.tile([128, 128], F16, tag="ident16")
            nc.vector.tensor_copy(out=ident16[:], in_=ident[:])
            ones1 = cpool.tile([1, 128], F16, tag="ones1")
            nc.vector.memset(ones1[:], 1.0)
            wd_t = [cpool.tile([128, DMID], F16, tag=f"wd{k}", name=f"wd{k}")
                    for k in range(2)]
            we_t = [cpool.tile([128, DMID], F16, tag=f"we{k}", name=f"we{k}")
                    for k in range(2)]
            wc_t = [cpool.tile([128, DOUT], F16, tag=f"wc{k}", name=f"wc{k}")
                    for k in range(2)]
            for k in range(2):
                nc.sync.dma_start(out=wd_t[k][:], in_=wd[k * 128:(k + 1) * 128, :])
                nc.sync.dma_start(out=we_t[k][:], in_=we[k * 128:(k + 1) * 128, :])
                nc.sync.dma_start(out=wc_t[k][:], in_=wc[k * 128:(k + 1) * 128, :])
            bd_t = cpool.tile([1, DMID], F16, tag="bd")
            be_t = cpool.tile([1, DMID], F8, tag="be")
            bc_t = cpool.tile([1, DOUT], F8, tag="bc")
            nc.gpsimd.dma_start(out=bd_t[:], in_=bd[:])
            nc.sync.dma_start(out=be_t[:], in_=be[:])
            nc.sync.dma_start(out=bc_t[:], in_=bc[:])

            # ---- degree vectors (host-computed) ----
            dinv = cpool.tile([128, NRT], F32, tag="dinv")
            nc.sync.dma_start(out=dinv[:], in_=dinv_d[:])
            dinv2 = cpool.tile([128, NRT], F32, tag="dinv2")
            nc.sync.dma_start(out=dinv2[:], in_=dinv2_d[:])
            sqdrow = cpool.tile([1, NRT * 128], F8, tag="sqdrow")
            nc.sync.dma_start(out=sqdrow[:], in_=sqdrow_d[:])
            nidx1_t = cpool.tile([1, m1["nidx"].shape[1]], I32, tag="nidx1")
            nc.sync.dma_start(out=nidx1_t[:], in_=nidx1[:])
            nidx2_t = cpool.tile([1, m2["nidx"].shape[1]], I32, tag="nidx2")
            nc.sync.dma_start(out=nidx2_t[:], in_=nidx2[:])
            nidx_regs = [nc.gpsimd.alloc_register(f"nidx_reg{c}")
                         for c in range(NCHUNK)]

            # zero both msg buffers once: trailing-trimmed gather slots leave
            # stale bytes which must be finite (they're multiplied by S=0)
            msgmax = max(max(g["tg"] for g in m1["sched"]),
                         2 * max(g["tg"] for g in m2["sched"])) * DMID
            for i in range(3):
                mz = gpool.tile([128, msgmax], F8, tag="msg", name=f"mz{i}",
                                bufs=3)
                nc.vector.memset(mz[:], 0.0)

            # ---- dense layer: u0 = relu(x @ wd + bd) * dinv ----
            # 4-tile batched loads/stores (fewer sequencer issues); batches
            # never cross AllGather piece boundaries.
            for p in range(4):
                t0p = int(pstart_t[p])
                for rt0 in range(t0p, t0p + ptiles[p], 4):
                    nt = min(4, t0p + ptiles[p] - rt0)
                    xt = wpool.tile([128, nt, DIN], F16, tag="xt")
                    nc.sync.dma_start(
                        out=xt[:],
                        in_=xtb[rt0 * 128:(rt0 + nt) * 128, :]
                        .rearrange("(t p) d -> p t d", p=128))
                    u0t = wpool.tile([128, nt * DMID], F8, tag="u0t")
                    for j in range(nt):
                        rt = rt0 + j
                        pu = ppool.tile([128, DMID], F32, tag="psu", bufs=3)
                        nc.tensor.matmul(out=pu[:], lhsT=xt[:, j, :128],
                                         rhs=wd_t[0][:],
                                         start=True, stop=False)
                        nc.tensor.matmul(out=pu[:], lhsT=xt[:, j, 128:],
                                         rhs=wd_t[1][:],
                                         start=False, stop=False)
                        nc.tensor.matmul(out=pu[:], lhsT=ones1[:],
                                         rhs=bd_t[:],
                                         start=False, stop=True)
                        nc.scalar.activation(
                            out=u0t[:, j * DMID:(j + 1) * DMID], in_=pu[:],
                            func=mybir.ActivationFunctionType.Relu,
                            scale=dinv[:, rt:rt + 1])
                    off = rt0 - t0p
                    nc.scalar.dma_start(
                        out=u0s[p][off * 128:(off + nt) * 128, :]
                        .rearrange("(t p) d -> p t d", p=128),
                        in_=u0t[:].rearrange("p (t d) -> p t d", d=DMID))
                nc.gpsimd.collective_compute(
                    "AllGather", mybir.AluOpType.bypass, replica_groups=rg,
                    ins=[u0s[p][:]], outs=[u0f[p][:]])

            def conv(m, sv_d, gidx_d, nidx_t, msg_dt, ident_s, u_full, u_selfs,
                     w_tiles, b_tile, out_writer, ag_after):
                pending = []
                ag_ready = []     # pieces whose last window's stage_b is out

                def stage_b(w, ps):
                    s1 = wpool.tile([128, DMID], F32, tag="s1")
                    nc.vector.tensor_copy(out=s1[:], in_=ps[:])
                    s1T = wpool.tile([128, DMID], F16, tag="s1T")
                    for h in range(2):
                        ptr = ppool.tile([128, 128], F32, tag="ptr", bufs=2)
                        nc.tensor.transpose(
                            out=ptr[:], in_=s1[:, h * 128:(h + 1) * 128],
                            identity=ident[:])
                        nc.vector.tensor_copy(
                            out=s1T[:, h * 128:(h + 1) * 128], in_=ptr[:])
                    pu = ppool.tile([128, DMID], F32, tag="psu", bufs=3)
                    nc.tensor.matmul(out=pu[:], lhsT=s1T[:, :128],
                                     rhs=w_tiles[0][:], start=True, stop=False)
                    nc.tensor.matmul(out=pu[:], lhsT=s1T[:, 128:],
                                     rhs=w_tiles[1][:], start=False, stop=False)
                    nc.tensor.matmul(out=pu[:],
                                     lhsT=sqdrow[:, w * 128:(w + 1) * 128],
                                     rhs=b_tile[:], start=False, stop=True)
                    out_writer(w, pu)
                    p, off = tile_piece(w)
                    if ag_after is not None and off + 1 == ptiles[p]:
                        # don't trigger here: the collective's wait would
                        # stall gathers queued behind it on the gpsimd queue
                        # until this window's store lands. Fire it one group
                        # later, when the store has long completed.
                        ag_ready.append(p)

                for g in m["sched"]:
                    ws, tg_g, nblk = g["ws"], g["tg"], g["nblk"]
                    base_msg, c_off = g["base_msg"], g["c_off"]
                    base_blk = g["base_blk"]
                    while ag_ready:
                        ag_after(ag_ready.pop(0))
                    msg = gpool.tile([128, tg_g * DMID], msg_dt, tag="msg",
                                     bufs=3)
                    sst = gpool.tile([128, nblk * 128], msg_dt, tag="sst")
                    gix = gpool.tile([128, tg_g * 8], I16, tag="gix", bufs=4)
                    nc.sync.dma_start(
                        out=gix[:],
                        in_=gidx_d[:, base_msg * 8:(base_msg + tg_g) * 8])
                    ci = 0
                    for c in range(NCHUNK):
                        tgc = (c_off[c + 1] if c + 1 < NCHUNK else tg_g) \
                            - c_off[c]
                        if tgc == 0:
                            continue
                        mo = c_off[c]
                        k = g["call_base"] + ci
                        ci += 1
                        nc.gpsimd.reg_load(nidx_regs[c], nidx_t[0:1, k:k + 1])
                        nc.gpsimd.dma_gather(
                            msg[:, mo * DMID:(mo + tgc) * DMID]
                                .rearrange("p (t d) -> p t d", d=DMID),
                            u_full[c][:],
                            gix[:, mo * 8:(mo + tgc) * 8],
                            num_idxs=tgc * 128,
                            num_idxs_reg=nidx_regs[c],
                            elem_size=DMID,
                            single_packet=False,
                            queue_num=c,
                        )
                    if msg_dt == F8:
                        nc.sync.dma_start(
                            out=sst[:],
                            in_=sv_d[:, base_blk * 128:(base_blk + nblk) * 128])
                    else:
                        # S stored fp8 in DRAM; DVE-convert to match msg dtype
                        sst8 = gpool.tile([128, nblk * 128], F8, tag="sst8")
                        nc.sync.dma_start(
                            out=sst8[:],
                            in_=sv_d[:, base_blk * 128:(base_blk + nblk) * 128])
                        nc.vector.tensor_copy(out=sst[:], in_=sst8[:])
                    uselfs = []
                    for w in ws:
                        ut = wpool.tile([128, DMID], msg_dt, tag="uself",
                                        bufs=9)
                        nc.scalar.dma_start(out=ut[:],
                                            in_=shard_rows(u_selfs, w))
                        uselfs.append(ut)
                    for wi, w in enumerate(ws):
                        ps = ppool.tile([128, DMID], F32, tag="pss", bufs=3)
                        blocks = g["win_blocks"][wi]
                        for k, (t, b) in enumerate(blocks):
                            nc.tensor.matmul(
                                out=ps[:],
                                lhsT=sst[:, b * 128:(b + 1) * 128],
                                rhs=msg[:, t * DMID:(t + 1) * DMID],
                                start=(k == 0), stop=False)
                        nc.tensor.matmul(out=ps[:], lhsT=ident_s[:],
                                         rhs=uselfs[wi][:],
                                         start=(len(blocks) == 0), stop=True)
                        pending.append((w, ps))
                        if len(pending) >= 2:
                            stage_b(*pending.pop(0))
                for w, ps in pending:
                    stage_b(w, ps)
                while ag_ready:
                    ag_after(ag_ready.pop(0))

            # conv1: u1 = dinv^2*(A@u0)@we + dinv*be   (pre-scaled by dinv)
            def write_u1(w, pu):
                u1t = wpool.tile([128, DMID], F8, tag="u1t")
                nc.scalar.activation(out=u1t[:], in_=pu[:],
                                     func=mybir.ActivationFunctionType.Copy,
                                     scale=dinv2[:, w:w + 1])
                nc.scalar.dma_start(out=shard_rows(u1s, w), in_=u1t[:])

            def ag_u1(p):
                nc.gpsimd.collective_compute(
                    "AllGather", mybir.AluOpType.bypass, replica_groups=rg,
                    ins=[u1s[p][:]], outs=[u1f[p][:]])

            conv(m1, sv1, gidx1, nidx1_t, F8, ident8, u0f, u0s,
                 we_t, be_t, write_u1, ag_u1)

            # conv2: out = dinv*(A@u1)@wc + bc
            def write_out(w, pu):
                ut = wpool.tile([128, DOUT], F32, tag="uout")
                nc.scalar.activation(out=ut[:], in_=pu[:],
                                     func=mybir.ActivationFunctionType.Copy,
                                     scale=dinv[:, w:w + 1])
                nc.scalar.dma_start(out=out[w * 128:(w + 1) * 128, :],
                                    in_=ut[:])

            conv(m2, sv2, gidx2, nidx2_t, F8, ident8, u1f, u1s,
                 wc_t, bc_t, write_out, None)

    nc.compile()
    return nc


def _run(inputs, cfg: Cfg, trace=False):
    x = inputs["x"]
    meta, data = _preprocess(x, inputs["edge_index"], inputs["edge_attr"], cfg)
    nc = _build_program(cfg, meta)

    wcat = np.concatenate([np.asarray(inputs["w_mu"], np.float32),
                           np.asarray(inputs["w_logstd"], np.float32)], axis=1)
    bcat = np.concatenate([np.asarray(inputs["b_mu"], np.float32),
                           np.asarray(inputs["b_logstd"], np.float32)])
    shared = dict(
        wd=np.asarray(inputs["w_dense"], np.float32).astype(NP_F16),
        bd=np.asarray(inputs["b_dense"], np.float32).reshape(1, -1),
        we=np.asarray(inputs["w_enc"], np.float32).astype(NP_F16),
        be=np.asarray(inputs["b_enc"], np.float32).reshape(1, -1)
        .astype(NP_F8),
        wc=wcat.astype(NP_F16), bc=bcat.reshape(1, -1).astype(NP_F8),
    )
    in_maps = []
    for c in range(cfg.NC):
        m = dict(shared)
        m["xtb"] = np.ascontiguousarray(data["xtb"][c])
        m["dinv_d"] = np.ascontiguousarray(data["dinv"][c])
        m["dinv2_d"] = np.ascontiguousarray(data["dinv2"][c])
        m["sqdrow_d"] = np.ascontiguousarray(data["sqdrow"][c])
        m["sv1"] = np.ascontiguousarray(meta["m1"]["sv"][c])
        m["gidx1"] = np.ascontiguousarray(meta["m1"]["gidx"][c])
        m["sv2"] = np.ascontiguousarray(meta["m2"]["sv"][c])
        m["gidx2"] = np.ascontiguousarray(meta["m2"]["gidx"][c])
        m["nidx1"] = np.ascontiguousarray(meta["m1"]["nidx"][c:c + 1])
        m["nidx2"] = np.ascontiguousarray(meta["m2"]["nidx"][c:c + 1])
        in_maps.append(m)

    res = run_bass_kernel_spmd(nc, in_maps, list(range(cfg.NC)), trace=trace)
    SH = cfg.SH
    halves = cfg.DOUT // 2
    mu = np.concatenate([res.results[c]["out"][:SH, :halves]
                         for c in range(cfg.NC)], axis=0)
    ls = np.concatenate([res.results[c]["out"][:SH, halves:]
                         for c in range(cfg.NC)], axis=0)
    return (mu.astype(np.float32), ls.astype(np.float32)), res


def kernel(**inputs):
    cfg = Cfg()
    (mu, ls), _ = _run(inputs, cfg, trace=False)
    return mu, ls


# revision 67
# speedup vs baseline: 1.3187x; 1.0046x over previous
"""GCN encoder (dense+relu -> GCNConv -> {mu, logstd} GCNConv) on 8 Trainium2
NeuronCores.

Strategy (v2):
  - Nodes sharded across 8 cores (12500 rows each, padded to 12544 = 98*128).
  - All message traffic in fp8e4 (gathered rows 256B/edge, scatter matrices
    S, AllGathered tables) for both convs; PSUM accumulation is f32, dense
    transforms f16. Measured rel err 1.20e-2 vs the 2e-2 gate (deterministic:
    hardware matches the numpy emulator to ~6 digits).
  - Edges partitioned by (dest group of GW windows, source chunk); slots
    padded to 128 per (group, chunk) cell only. A tile whose 128 edges span
    several dest windows gets one S block per window it touches (union span
    across cores so the schedule is core-independent). conv1 GW=7, conv2
    GW=2 (f16 tiles are 2x bigger in SBUF).
  - Pad slots carry idx -1 (trailing within each gather call) and the true
    per-core valid count is fed via a runtime register, so the Q7 gather
    ucode skips them entirely (its time is the kernel's hard floor at
    ~2.9ns per real edge, ~1.23ms total).
  - u tables AllGathered in 4 window-aligned pieces (25/25/24/24 row tiles)
    so the next conv's gathers start as soon as their chunk's piece lands.
    AG triggers are deferred one group so their waits never stall gathers
    queued behind them on the gpsimd queue.
  - Degree vectors (dinv, dinv^2, sqrt(deg)) are host-computed inputs; bias
    adds use a rank-1 matmul with the sqrt(deg) row so the per-partition
    output scale folds to the right per-term factors.
  - Dense layer uses host-pretransposed f16 lhsT tiles (no PE transposes),
    4-tile batched loads/stores.
  - Window loop software-pipelined (scatter of w+1 issued before transform
    of w) so PE doesn't stall on the PSUM->SBUF copy round trip; self-loop
    matmul goes last so its DRAM load hides behind the scatter matmuls.
"""
import sys

sys.path.insert(0, "/opt/trn_rl_repo")

import numpy as np
import ml_dtypes

import concourse.bacc as bacc
import concourse.bass as bass
import concourse.mybir as mybir
from concourse import tile
from concourse.bass_utils import run_bass_kernel_spmd
from concourse.masks import make_identity

F32 = mybir.dt.float32
F16 = mybir.dt.float16
F8 = mybir.dt.float8e4
I16 = mybir.dt.int16
NP_F8 = ml_dtypes.float8_e4m3
NP_F16 = np.float16
PAD_TRIM = True
I32 = mybir.dt.int32


class Cfg:
    def __init__(self, N=100000, NC=8, DIN=256, DMID=256, DOUT=256,
                 GW1=7, GW2=7):
        assert N % NC == 0
        self.N, self.NC = N, NC
        self.DIN, self.DMID, self.DOUT = DIN, DMID, DOUT
        self.SH = N // NC                       # real rows per shard
        self.SHP = -(-self.SH // 128) * 128     # padded rows per shard
        self.W = 128                            # dest window size
        self.NW = self.SHP // self.W            # windows (= row tiles)
        self.GW1, self.GW2 = GW1, GW2
        # 4 window-aligned AllGather pieces; piece == gather chunk
        q, r = divmod(self.NW, 4)
        self.piece_tiles = [q + (1 if i < r else 0) for i in range(4)]
        self.piece_rows = [t * 128 for t in self.piece_tiles]
        self.piece_starts = np.concatenate(
            [[0], np.cumsum(self.piece_rows)]).astype(np.int64)
        self.NCHUNK = 4
        self.CH = [NC * r for r in self.piece_rows]   # rows per gather chunk
        assert max(self.CH) <= 32767


def _edge_schedule(core, ldst, cidx, chunk, ew, cfg: Cfg, GW, npdt):
    """Core-independent tile/block schedule + per-core S and idx tables."""
    NC, W, NW, NCHUNK = cfg.NC, cfg.W, cfg.NW, cfg.NCHUNK
    NG = -(-NW // GW)
    win = ldst // W
    grp = win // GW
    wig = win - grp * GW
    dstoff = ldst - win * W

    cell = (core * NG + grp) * NCHUNK + chunk
    order = np.lexsort((cidx, wig, cell))
    cell_s = cell[order]
    n_cells = NC * NG * NCHUNK
    counts = np.bincount(cell, minlength=n_cells).reshape(NC, NG, NCHUNK)

    T_gc = -(-counts.max(axis=0) // 128)            # [NG, NCHUNK]
    c_off = np.zeros((NG, NCHUNK), np.int64)
    c_off[:, 1:] = np.cumsum(T_gc, axis=1)[:, :-1]
    tg = T_gc.sum(axis=1)
    base_g = np.zeros(NG, np.int64)
    base_g[1:] = np.cumsum(tg)[:-1]
    TOT = int(tg.sum())
    tile_base = base_g[:, None] + c_off

    starts = np.zeros(n_cells + 1, np.int64)
    starts[1:] = np.cumsum(counts.reshape(-1))
    rank_s = np.arange(len(cell_s), dtype=np.int64) - starts[cell_s]
    g_s = (cell_s // NCHUNK) % NG
    c_s = cell_s % NCHUNK
    core_s = cell_s // (NG * NCHUNK)
    erow_s = (tile_base[g_s, c_s] + rank_s // 128) * 128 + rank_s % 128

    # pad slots get idx -1: they are trailing within each (g,c) gather call,
    # and the Q7 gather kernel trims trailing negative indices (no descriptor
    # work, no DMA bytes). Their S entries are 0 so stale msg data is killed.
    PAD_IDX = -1 if PAD_TRIM else 0
    IDXRAW = np.full((NC, TOT * 128), PAD_IDX, np.int16)
    IDXRAW[core_s, erow_s] = cidx[order].astype(np.int16)
    idxg = IDXRAW.reshape(NC, TOT * 8, 16).transpose(0, 2, 1)
    IDXG = np.tile(idxg, (1, 8, 1))                 # [NC, 128, TOT*8]

    key4 = cell * GW + wig
    counts4 = np.bincount(key4, minlength=n_cells * GW) \
        .reshape(NC, NG, NCHUNK, GW)
    ends4 = np.cumsum(counts4, axis=3)
    starts4 = ends4 - counts4
    has = counts4 > 0
    T_LO = np.where(has, starts4 // 128, 1 << 30).min(axis=0)
    T_HI = np.where(has, (ends4 - 1) // 128, -1).max(axis=0)
    present = has.any(axis=0)

    blk_start = np.full((NG, NCHUNK, GW), -1, np.int64)
    base_blk = np.zeros(NG, np.int64)
    sched = []
    nblk_total = 0
    nidx_cols = []                     # per gather call: per-core valid count
    for g in range(NG):
        ws = list(range(g * GW, min((g + 1) * GW, NW)))
        base_blk[g] = nblk_total
        call_base = len(nidx_cols)
        for c in range(NCHUNK):
            if T_gc[g, c] > 0:
                nidx_cols.append(counts[:, g, c])
        win_blocks = []
        bi = 0
        for wi in range(len(ws)):
            blocks = []
            for c in range(NCHUNK):
                if not present[g, c, wi]:
                    continue
                blk_start[g, c, wi] = bi
                for t in range(int(T_LO[g, c, wi]), int(T_HI[g, c, wi]) + 1):
                    blocks.append((int(c_off[g, c] + t), bi))
                    bi += 1
            win_blocks.append(blocks)
        nblk_total += bi
        sched.append(dict(ws=ws, base_msg=int(base_g[g]), tg=int(tg[g]),
                          c_off=[int(v) for v in c_off[g]],
                          base_blk=int(base_blk[g]), nblk=bi,
                          call_base=call_base,
                          win_blocks=win_blocks))
    B_TOT = nblk_total
    NIDX = np.stack(nidx_cols, axis=1).astype(np.int32)   # [NC, ncalls]

    t_in_cell = rank_s // 128
    wig_s = wig[order]
    blk_s = (base_blk[g_s] + blk_start[g_s, c_s, wig_s]
             + (t_in_cell - T_LO[g_s, c_s, wig_s]))
    Sv = np.zeros((NC, 128, B_TOT * 128), npdt)
    Sv[core_s, rank_s % 128, blk_s * 128 + dstoff[order]] = \
        ew[order].astype(npdt)

    return dict(TOT=TOT, B_TOT=B_TOT, sched=sched, sv=Sv, gidx=IDXG,
                nidx=NIDX)


def _preprocess(x, edge_index, edge_attr, cfg: Cfg):
    NC, SH, SHP = cfg.NC, cfg.SH, cfg.SHP

    src = np.asarray(edge_index[0], dtype=np.int64)
    dst = np.asarray(edge_index[1], dtype=np.int64)
    ew = np.asarray(edge_attr, dtype=np.float32)
    core = dst // SH
    ldst = dst - core * SH
    s_shard = src // SH
    s_loc = src - s_shard * SH
    s_piece = np.searchsorted(cfg.piece_starts, s_loc, side="right") - 1
    prow = np.asarray(cfg.piece_rows, np.int64)
    cidx = s_shard * prow[s_piece] + (s_loc - cfg.piece_starts[s_piece])

    m1 = _edge_schedule(core, ldst, cidx, s_piece, ew, cfg, cfg.GW1, NP_F8)
    m2 = _edge_schedule(core, ldst, cidx, s_piece, ew, cfg, cfg.GW2, NP_F8)

    # degree (with self-loop weight 1) computed on host
    deg = np.bincount(core * SHP + ldst, weights=ew.astype(np.float64),
                      minlength=NC * SHP).reshape(NC, SHP) \
        .astype(np.float32) + 1.0
    dinv2 = 1.0 / deg
    dinv = np.sqrt(dinv2)
    sqd = np.sqrt(deg)
    NW = cfg.NW
    # [NC, 128, NW] layout: [c, p, rt] = value at row rt*128+p
    dinv_t = dinv.reshape(NC, NW, 128).transpose(0, 2, 1).copy()
    dinv2_t = dinv2.reshape(NC, NW, 128).transpose(0, 2, 1).copy()
    sqdrow = sqd.reshape(NC, 1, SHP).astype(NP_F8)

    # dense lhsT tiles, host-transposed, f16:
    # xtb[c, rt*128+p, h*128+j] = x[c*SH + rt*128 + j, h*128 + p]
    xs = np.zeros((NC, SHP, cfg.DIN), np.float32)
    xs[:, :SH, :] = np.asarray(x, np.float32).reshape(NC, SH, cfg.DIN)
    xtb = xs.reshape(NC, NW, 128, cfg.DIN).transpose(0, 1, 3, 2) \
        .reshape(NC, NW, 2, 128, 128).transpose(0, 1, 3, 2, 4) \
        .reshape(NC, NW * 128, cfg.DIN).astype(NP_F16)

    meta = dict(m1=m1, m2=m2)
    data = dict(xtb=xtb, dinv=dinv_t, dinv2=dinv2_t, sqdrow=sqdrow)
    return meta, data


def _build_program(cfg: Cfg, meta):
    NC, SHP, W, NW = cfg.NC, cfg.SHP, cfg.W, cfg.NW
    DIN, DMID, DOUT = cfg.DIN, cfg.DMID, cfg.DOUT
    NCHUNK = cfg.NCHUNK
    m1, m2 = meta["m1"], meta["m2"]
    NRT = NW
    ptiles = cfg.piece_tiles
    pstart_t = np.concatenate([[0], np.cumsum(ptiles)])

    nc = bacc.Bacc("TRN2", target_bir_lowering=False, debug=False,
                   num_devices=NC, num_swdge_queues=4)

    xtb = nc.dram_tensor("xtb", [SHP, DIN], F16, kind="ExternalInput")
    dinv_d = nc.dram_tensor("dinv_d", [128, NRT], F32, kind="ExternalInput")
    dinv2_d = nc.dram_tensor("dinv2_d", [128, NRT], F32, kind="ExternalInput")
    sqdrow_d = nc.dram_tensor("sqdrow_d", [1, SHP], F8, kind="ExternalInput")
    sv1 = nc.dram_tensor("sv1", [128, m1["B_TOT"] * 128], F8,
                         kind="ExternalInput")
    gidx1 = nc.dram_tensor("gidx1", [128, m1["TOT"] * 8], I16,
                           kind="ExternalInput")
    sv2 = nc.dram_tensor("sv2", [128, m2["B_TOT"] * 128], F8,
                         kind="ExternalInput")
    gidx2 = nc.dram_tensor("gidx2", [128, m2["TOT"] * 8], I16,
                           kind="ExternalInput")
    nidx1 = nc.dram_tensor("nidx1", [1, m1["nidx"].shape[1]], I32,
                           kind="ExternalInput")
    nidx2 = nc.dram_tensor("nidx2", [1, m2["nidx"].shape[1]], I32,
                           kind="ExternalInput")
    wd = nc.dram_tensor("wd", [DIN, DMID], F16, kind="ExternalInput")
    bd = nc.dram_tensor("bd", [1, DMID], F32, kind="ExternalInput")
    we = nc.dram_tensor("we", [DMID, DMID], F16, kind="ExternalInput")
    be = nc.dram_tensor("be", [1, DMID], F8, kind="ExternalInput")
    wc = nc.dram_tensor("wc", [DMID, DOUT], F16, kind="ExternalInput")
    bc = nc.dram_tensor("bc", [1, DOUT], F8, kind="ExternalInput")
    out = nc.dram_tensor("out", [SHP, DOUT], F32, kind="ExternalOutput")
    u0s = [nc.dram_tensor(f"u0s{p}", [cfg.piece_rows[p], DMID], F8)
           for p in range(4)]
    u0f = [nc.dram_tensor(f"u0f{p}", [cfg.CH[p], DMID], F8,
                          addr_space="Shared") for p in range(4)]
    u1s = [nc.dram_tensor(f"u1s{p}", [cfg.piece_rows[p], DMID], F8)
           for p in range(4)]
    u1f = [nc.dram_tensor(f"u1f{p}", [cfg.CH[p], DMID], F8,
                          addr_space="Shared") for p in range(4)]

    rg = [list(range(NC))]

    def tile_piece(rt):
        p = int(np.searchsorted(pstart_t, rt, side="right") - 1)
        return p, rt - int(pstart_t[p])

    def shard_rows(dram_list, rt):
        p, off = tile_piece(rt)
        return dram_list[p][off * 128:(off + 1) * 128, :]

    with tile.TileContext(nc) as tc:
        with (
            tc.tile_pool(name="const", bufs=1) as cpool,
            tc.tile_pool(name="work", bufs=4) as wpool,
            tc.tile_pool(name="spmm", bufs=2) as gpool,
            tc.tile_pool(name="psum", bufs=2, space="PSUM") as ppool,
        ):
            # ---- constants ----
            ident = cpool.tile([128, 128], F32, tag="ident")
            make_identity(nc, ident[:])
            ident8 = cpool.tile([128, 128], F8, tag="ident8")
            nc.vector.tensor_copy(out=ident8[:], in_=ident[:])
            ident16 = cpool.tile([128, 128], F16, tag="ident16")
            nc.vector.tensor_copy(out=ident16[:], in_=ident[:])
            ones1 = cpool.tile([1, 128], F16, tag="ones1")
            nc.vector.memset(ones1[:], 1.0)
            wd_t = [cpool.tile([128, DMID], F16, tag=f"wd{k}", name=f"wd{k}")
                    for k in range(2)]
            we_t = [cpool.tile([128, DMID], F16, tag=f"we{k}", name=f"we{k}")
                    for k in range(2)]
            wc_t = [cpool.tile([128, DOUT], F16, tag=f"wc{k}", name=f"wc{k}")
                    for k in range(2)]
            for k in range(2):
                nc.sync.dma_start(out=wd_t[k][:], in_=wd[k * 128:(k + 1) * 128, :])
                nc.sync.dma_start(out=we_t[k][:], in_=we[k * 128:(k + 1) * 128, :])
                nc.sync.dma_start(out=wc_t[k][:], in_=wc[k * 128:(k + 1) * 128, :])
            bd_t = cpool.tile([1, DMID], F16, tag="bd")
            be_t = cpool.tile([1, DMID], F8, tag="be")
            bc_t = cpool.tile([1, DOUT], F8, tag="bc")
            nc.gpsimd.dma_start(out=bd_t[:], in_=bd[:])
            nc.sync.dma_start(out=be_t[:], in_=be[:])
            nc.sync.dma_start(out=bc_t[:], in_=bc[:])

            # ---- degree vectors (host-computed) ----
            dinv = cpool.tile([128, NRT], F32, tag="dinv")
            nc.sync.dma_start(out=dinv[:], in_=dinv_d[:])
            dinv2 = cpool.tile([128, NRT], F32, tag="dinv2")
            nc.sync.dma_start(out=dinv2[:], in_=dinv2_d[:])
            sqdrow = cpool.tile([1, NRT * 128], F8, tag="sqdrow")
            nc.sync.dma_start(out=sqdrow[:], in_=sqdrow_d[:])
            nidx1_t = cpool.tile([1, m1["nidx"].shape[1]], I32, tag="nidx1")
            nc.sync.dma_start(out=nidx1_t[:], in_=nidx1[:])
            nidx2_t = cpool.tile([1, m2["nidx"].shape[1]], I32, tag="nidx2")
            nc.sync.dma_start(out=nidx2_t[:], in_=nidx2[:])
            nidx_regs = [nc.gpsimd.alloc_register(f"nidx_reg{c}")
                         for c in range(NCHUNK)]

            # zero both msg buffers once: trailing-trimmed gather slots leave
            # stale bytes which must be finite (they're multiplied by S=0)
            # both convs' msg tiles are fp8 now: 1 byte per element
            msgmax = max(max(g["tg"] for g in m1["sched"]),
                         max(g["tg"] for g in m2["sched"])) * DMID
            for i in range(3):
                mz = gpool.tile([128, msgmax], F8, tag="msg", name=f"mz{i}",
                                bufs=3)
                nc.vector.memset(mz[:], 0.0)

            # ---- dense layer: u0 = relu(x @ wd + bd) * dinv ----
            # 4-tile batched loads/stores (fewer sequencer issues); batches
            # never cross AllGather piece boundaries.
            for p in range(4):
                t0p = int(pstart_t[p])
                for rt0 in range(t0p, t0p + ptiles[p], 4):
                    nt = min(4, t0p + ptiles[p] - rt0)
                    xt = wpool.tile([128, nt, DIN], F16, tag="xt")
                    nc.sync.dma_start(
                        out=xt[:],
                        in_=xtb[rt0 * 128:(rt0 + nt) * 128, :]
                        .rearrange("(t p) d -> p t d", p=128))
                    u0t = wpool.tile([128, nt * DMID], F8, tag="u0t")
                    for j in range(nt):
                        rt = rt0 + j
                        pu = ppool.tile([128, DMID], F32, tag="psu", bufs=3)
                        nc.tensor.matmul(out=pu[:], lhsT=xt[:, j, :128],
                                         rhs=wd_t[0][:],
                                         start=True, stop=False)
                        nc.tensor.matmul(out=pu[:], lhsT=xt[:, j, 128:],
                                         rhs=wd_t[1][:],
                                         start=False, stop=False)
                        nc.tensor.matmul(out=pu[:], lhsT=ones1[:],
                                         rhs=bd_t[:],
                                         start=False, stop=True)
                        nc.scalar.activation(
                            out=u0t[:, j * DMID:(j + 1) * DMID], in_=pu[:],
                            func=mybir.ActivationFunctionType.Relu,
                            scale=dinv[:, rt:rt + 1])
                    off = rt0 - t0p
                    nc.scalar.dma_start(
                        out=u0s[p][off * 128:(off + nt) * 128, :]
                        .rearrange("(t p) d -> p t d", p=128),
                        in_=u0t[:].rearrange("p (t d) -> p t d", d=DMID))
                nc.gpsimd.collective_compute(
                    "AllGather", mybir.AluOpType.bypass, replica_groups=rg,
                    ins=[u0s[p][:]], outs=[u0f[p][:]])

            def conv(m, sv_d, gidx_d, nidx_t, msg_dt, ident_s, u_full, u_selfs,
                     w_tiles, b_tile, out_writer, ag_after):
                pending = []
                ag_ready = []     # pieces whose last window's stage_b is out

                def stage_b(w, ps):
                    s1 = wpool.tile([128, DMID], F32, tag="s1")
                    nc.vector.tensor_copy(out=s1[:], in_=ps[:])
                    s1T = wpool.tile([128, DMID], F16, tag="s1T")
                    for h in range(2):
                        ptr = ppool.tile([128, 128], F32, tag="ptr", bufs=2)
                        nc.tensor.transpose(
                            out=ptr[:], in_=s1[:, h * 128:(h + 1) * 128],
                            identity=ident[:])
                        nc.vector.tensor_copy(
                            out=s1T[:, h * 128:(h + 1) * 128], in_=ptr[:])
                    pu = ppool.tile([128, DMID], F32, tag="psu", bufs=3)
                    nc.tensor.matmul(out=pu[:], lhsT=s1T[:, :128],
                                     rhs=w_tiles[0][:], start=True, stop=False)
                    nc.tensor.matmul(out=pu[:], lhsT=s1T[:, 128:],
                                     rhs=w_tiles[1][:], start=False, stop=False)
                    nc.tensor.matmul(out=pu[:],
                                     lhsT=sqdrow[:, w * 128:(w + 1) * 128],
                                     rhs=b_tile[:], start=False, stop=True)
                    out_writer(w, pu)
                    p, off = tile_piece(w)
                    if ag_after is not None and off + 1 == ptiles[p]:
                        # don't trigger here: the collective's wait would
                        # stall gathers queued behind it on the gpsimd queue
                        # until this window's store lands. Fire it one group
                        # later, when the store has long completed.
                        ag_ready.append(p)

                for g in m["sched"]:
                    ws, tg_g, nblk = g["ws"], g["tg"], g["nblk"]
                    base_msg, c_off = g["base_msg"], g["c_off"]
                    base_blk = g["base_blk"]
                    while ag_ready:
                        ag_after(ag_ready.pop(0))
                    msg = gpool.tile([128, tg_g * DMID], msg_dt, tag="msg",
                                     bufs=3)
                    sst = gpool.tile([128, nblk * 128], msg_dt, tag="sst")
                    gix = gpool.tile([128, tg_g * 8], I16, tag="gix", bufs=4)
                    nc.sync.dma_start(
                        out=gix[:],
                        in_=gidx_d[:, base_msg * 8:(base_msg + tg_g) * 8])
                    ci = 0
                    for c in range(NCHUNK):
                        tgc = (c_off[c + 1] if c + 1 < NCHUNK else tg_g) \
                            - c_off[c]
                        if tgc == 0:
                            continue
                        mo = c_off[c]
                        k = g["call_base"] + ci
                        ci += 1
                        nc.gpsimd.reg_load(nidx_regs[c], nidx_t[0:1, k:k + 1])
                        nc.gpsimd.dma_gather(
                            msg[:, mo * DMID:(mo + tgc) * DMID]
                                .rearrange("p (t d) -> p t d", d=DMID),
                            u_full[c][:],
                            gix[:, mo * 8:(mo + tgc) * 8],
                            num_idxs=tgc * 128,
                            num_idxs_reg=nidx_regs[c],
                            elem_size=DMID,
                            single_packet=False,
                            queue_num=c,
                        )
                    if msg_dt == F8:
                        nc.sync.dma_start(
                            out=sst[:],
                            in_=sv_d[:, base_blk * 128:(base_blk + nblk) * 128])
                    else:
                        # S stored fp8 in DRAM; DVE-convert to match msg dtype
                        sst8 = gpool.tile([128, nblk * 128], F8, tag="sst8")
                        nc.sync.dma_start(
                            out=sst8[:],
                            in_=sv_d[:, base_blk * 128:(base_blk + nblk) * 128])
                        nc.vector.tensor_copy(out=sst[:], in_=sst8[:])
                    uselfs = []
                    for w in ws:
                        ut = wpool.tile([128, DMID], msg_dt, tag="uself",
                                        bufs=9)
                        nc.scalar.dma_start(out=ut[:],
                                            in_=shard_rows(u_selfs, w))
                        uselfs.append(ut)
                    for wi, w in enumerate(ws):
                        ps = ppool.tile([128, DMID], F32, tag="pss", bufs=3)
                        blocks = g["win_blocks"][wi]
                        for k, (t, b) in enumerate(blocks):
                            nc.tensor.matmul(
                                out=ps[:],
                                lhsT=sst[:, b * 128:(b + 1) * 128],
                                rhs=msg[:, t * DMID:(t + 1) * DMID],
                                start=(k == 0), stop=False)
                        nc.tensor.matmul(out=ps[:], lhsT=ident_s[:],
                                         rhs=uselfs[wi][:],
                                         start=(len(blocks) == 0), stop=True)
                        pending.append((w, ps))
                        if len(pending) >= 2:
                            stage_b(*pending.pop(0))
                for w, ps in pending:
                    stage_b(w, ps)
                while ag_ready:
                    ag_after(ag_ready.pop(0))

            # conv1: u1 = dinv^2*(A@u0)@we + dinv*be   (pre-scaled by dinv)
            def write_u1(w, pu):
                u1t = wpool.tile([128, DMID], F8, tag="u1t")
                nc.scalar.activation(out=u1t[:], in_=pu[:],
                                     func=mybir.ActivationFunctionType.Copy,
                                     scale=dinv2[:, w:w + 1])
                nc.scalar.dma_start(out=shard_rows(u1s, w), in_=u1t[:])

            def ag_u1(p):
                nc.gpsimd.collective_compute(
                    "AllGather", mybir.AluOpType.bypass, replica_groups=rg,
                    ins=[u1s[p][:]], outs=[u1f[p][:]])

            conv(m1, sv1, gidx1, nidx1_t, F8, ident8, u0f, u0s,
                 we_t, be_t, write_u1, ag_u1)

            # conv2: out = dinv*(A@u1)@wc + bc
            def write_out(w, pu):
                ut = wpool.tile([128, DOUT], F32, tag="uout")
                nc.scalar.activation(out=ut[:], in_=pu[:],
                                     func=mybir.ActivationFunctionType.Copy,
                                     scale=dinv[:, w:w + 1])
                nc.scalar.dma_start(out=out[w * 128:(w + 1) * 128, :],
                                    in_=ut[:])

            conv(m2, sv2, gidx2, nidx2_t, F8, ident8, u1f, u1s,
                 wc_t, bc_t, write_out, None)

    nc.compile()
    return nc


def _run(inputs, cfg: Cfg, trace=False):
    x = inputs["x"]
    meta, data = _preprocess(x, inputs["edge_index"], inputs["edge_attr"], cfg)
    nc = _build_program(cfg, meta)

    wcat = np.concatenate([np.asarray(inputs["w_mu"], np.float32),
                           np.asarray(inputs["w_logstd"], np.float32)], axis=1)
    bcat = np.concatenate([np.asarray(inputs["b_mu"], np.float32),
                           np.asarray(inputs["b_logstd"], np.float32)])
    shared = dict(
        wd=np.asarray(inputs["w_dense"], np.float32).astype(NP_F16),
        bd=np.asarray(inputs["b_dense"], np.float32).reshape(1, -1),
        we=np.asarray(inputs["w_enc"], np.float32).astype(NP_F16),
        be=np.asarray(inputs["b_enc"], np.float32).reshape(1, -1)
        .astype(NP_F8),
        wc=wcat.astype(NP_F16), bc=bcat.reshape(1, -1).astype(NP_F8),
    )
    in_maps = []
    for c in range(cfg.NC):
        m = dict(shared)
        m["xtb"] = np.ascontiguousarray(data["xtb"][c])
        m["dinv_d"] = np.ascontiguousarray(data["dinv"][c])
        m["dinv2_d"] = np.ascontiguousarray(data["dinv2"][c])
        m["sqdrow_d"] = np.ascontiguousarray(data["sqdrow"][c])
        m["sv1"] = np.ascontiguousarray(meta["m1"]["sv"][c])
        m["gidx1"] = np.ascontiguousarray(meta["m1"]["gidx"][c])
        m["sv2"] = np.ascontiguousarray(meta["m2"]["sv"][c])
        m["gidx2"] = np.ascontiguousarray(meta["m2"]["gidx"][c])
        m["nidx1"] = np.ascontiguousarray(meta["m1"]["nidx"][c:c + 1])
        m["nidx2"] = np.ascontiguousarray(meta["m2"]["nidx"][c:c + 1])
        in_maps.append(m)

    res = run_bass_kernel_spmd(nc, in_maps, list(range(cfg.NC)), trace=trace)
    SH = cfg.SH
    halves = cfg.DOUT // 2
    mu = np.concatenate([res.results[c]["out"][:SH, :halves]
                         for c in range(cfg.NC)], axis=0)
    ls = np.concatenate([res.results[c]["out"][:SH, halves:]
                         for c in range(cfg.NC)], axis=0)
    return (mu.astype(np.float32), ls.astype(np.float32)), res


def kernel(**inputs):
    cfg = Cfg()
    (mu, ls), _ = _run(inputs, cfg, trace=False)
    return mu, ls
